# revision 8
# baseline (speedup 1.0000x reference)
"""Trainium2 Bass kernel: GNN attention message-passing (AMP layer).

reference math (per node n, K neighbors):
    q      = nodes @ wq                       [N, FE]
    rq     = q @ wk.T = nodes @ (wq @ wk.T)   [N, FE]   (host folds wq@wk.T)
    logit[n,k] = inv_degree[n] * (edges[n,k,:] . rq[n,:])
    b      = softmax_k(logit)
    agg[n] = sum_k b[n,k] * nodes[nlist[n,k]]
    out    = agg @ wv

Distribution: node axis N sharded over 8 cores (6250 rows each, padded to
6272 = 49 tiles of 128). The full nodes table is replicated into every
core's DRAM; the neighbor gather is a per-core dma_gather. No collectives.

The gather uses int16 indices (hardware constraint), which cannot address
50000 rows directly, so the table is viewed as 25000 PAIR tokens of 2x128
floats and idx = nlist//2; the wrong half of each gathered pair is masked
out in the weighted-reduction coefficient matrix (even/odd split).

Per 128-node tile on each core:
  - dma_gather 4096 pair tokens (single_packet=False: >64 desc/engine)
  - logits/softmax on DVE+ACT (per-partition grouped dot products)
  - weighted neighbor reduction as 2x32 small PE matmuls against even/odd
    block-diagonal coefficient matrices built on-chip from softmax output
  - final projection by wv on PE
"""

from contextlib import ExitStack

import numpy as np

import concourse.bass as bass
import concourse.bacc as bacc
import concourse.tile as tile
from concourse import mybir
from concourse.bass_utils import run_bass_kernel_spmd

N, K, FN, FE = 50000, 32, 128, 64
NCORES = 8
SH = N // NCORES            # rows per core (6250)
TILE = 128
NT = -(-SH // TILE)         # tiles per core (49)
PAD = NT * TILE             # padded rows per core (6272)
CPG = TILE // K             # nodes completed per gather block (4)
NIDX = TILE * K             # gathered rows per tile (4096)
NW = NIDX // 16             # wrapped idx columns (256)

F32 = mybir.dt.float32
I16 = mybir.dt.int16

_CACHE: dict = {}


def _build_nc(n_table: int | None = None, nt: int | None = None):
    """Build the SPMD per-core graph. Identical on all 8 cores; only the
    DRAM input contents differ per core."""
    n_table = N if n_table is None else n_table
    nt = NT if nt is None else nt
    pad = nt * TILE
    npair = n_table // 2
    nc = bacc.Bacc()

    nodes_d = nc.dram_tensor("nodes", [npair, 2 * FN], F32, kind="ExternalInput")
    xselfT_d = nc.dram_tensor("xselfT", [nt, FN, TILE], F32, kind="ExternalInput")
    edges_d = nc.dram_tensor("edges", [pad, K, FE], F32, kind="ExternalInput")
    pidx_d = nc.dram_tensor("pidx", [nt, 128, NW], I16, kind="ExternalInput")
    par_d = nc.dram_tensor("par", [nt, TILE, K], F32, kind="ExternalInput")
    inv_d = nc.dram_tensor("invdeg", [pad, 1], F32, kind="ExternalInput")
    wv_d = nc.dram_tensor("wv", [FN, FN], F32, kind="ExternalInput")
    wqkt_d = nc.dram_tensor("wqkt", [FN, FE], F32, kind="ExternalInput")
    m128_d = nc.dram_tensor("m128", [TILE, TILE], F32, kind="ExternalInput")
    i4t_d = nc.dram_tensor("i4t", [K, TILE], F32, kind="ExternalInput")
    ident_d = nc.dram_tensor("ident", [TILE, TILE], F32, kind="ExternalInput")
    out_d = nc.dram_tensor("out", [pad, FN], F32, kind="ExternalOutput")

    with tile.TileContext(nc) as tc, ExitStack() as ctx:
        consts = ctx.enter_context(tc.tile_pool(name="consts", bufs=1))
        big = ctx.enter_context(tc.tile_pool(name="big", bufs=3))
        gat = ctx.enter_context(tc.tile_pool(name="gat", bufs=2))
        med = ctx.enter_context(tc.tile_pool(name="med", bufs=3))
        small = ctx.enter_context(tc.tile_pool(name="small", bufs=4))
        psum = ctx.enter_context(tc.tile_pool(name="psum", bufs=1, space="PSUM"))

        wv_sb = consts.tile([FN, FN], F32)
        nc.sync.dma_start(wv_sb[:], wv_d[:, :])
        wqkt_sb = consts.tile([FN, FE], F32)
        nc.sync.dma_start(wqkt_sb[:], wqkt_d[:, :])
        m128_sb = consts.tile([TILE, TILE], F32)
        nc.sync.dma_start(m128_sb[:], m128_d[:, :])
        i4t_sb = consts.tile([K, TILE], F32)
        nc.sync.dma_start(i4t_sb[:], i4t_d[:, :])
        ident_sb = consts.tile([TILE, TILE], F32)
        nc.sync.dma_start(ident_sb[:], ident_d[:, :])

        for t in range(nt):
            r0 = t * TILE

            # pair-token gather: element i lands at xg[i%128, i//128, :]
            pidx = small.tile([128, NW], I16, tag="pidx")
            nc.sync.dma_start(pidx[:], pidx_d[t, :, :])
            xg = gat.tile([TILE, K, 2 * FN], F32, tag="xg")
            nc.gpsimd.dma_gather(
                xg[:], nodes_d[:, :], pidx[:],
                num_idxs=NIDX, num_idxs_reg=NIDX, elem_size=2 * FN,
                single_packet=False,
            )

            # self features (pre-transposed on host): xsT[f, n]
            xsT = med.tile([FN, TILE], F32, tag="xsT")
            nc.sync.dma_start(xsT[:], xselfT_d[t, :, :])

            # rq[n, c] = sum_f xself[n, f] * (wq@wk.T)[f, c]
            rq_ps = psum.tile([TILE, FE], F32, tag="rq_ps")
            nc.tensor.matmul(rq_ps[:], lhsT=xsT[:], rhs=wqkt_sb[:])
            rq = small.tile([TILE, FE], F32, tag="rq")
            nc.scalar.copy(rq[:], rq_ps[:])

            # edges tile + logits: dots[n, k] = sum_c edges[n,k,c] * rq[n,c]
            ed = big.tile([TILE, K, FE], F32, tag="ed")
            nc.sync.dma_start(ed[:], edges_d[r0:r0 + TILE, :, :])
            prod = big.tile([TILE, K, FE], F32, tag="prod")
            rq_ap = rq[:]
            rq_bc = bass.AP(
                tensor=rq_ap.tensor,
                offset=rq_ap.offset,
                ap=[rq_ap.ap[0], [0, K], rq_ap.ap[1]],
            )
            nc.vector.tensor_tensor(
                out=prod[:], in0=ed[:], in1=rq_bc, op=mybir.AluOpType.mult
            )
            dots = small.tile([TILE, K], F32, tag="dots")
            nc.vector.tensor_reduce(
                out=dots[:], in_=prod[:], axis=mybir.AxisListType.X,
                op=mybir.AluOpType.add,
            )

            # scale by inv_degree, softmax over k (normalization deferred)
            iv = small.tile([TILE, 1], F32, tag="iv")
            nc.sync.dma_start(iv[:], inv_d[r0:r0 + TILE, :])
            scaled = small.tile([TILE, K], F32, tag="scaled")
            nc.vector.tensor_scalar_mul(scaled[:], dots[:], iv[:])
            negmax = small.tile([TILE, 1], F32, tag="negmax")
            nc.vector.reduce_max(
                out=negmax[:], in_=scaled[:], axis=mybir.AxisListType.X, negate=True
            )
            expb = small.tile([TILE, K], F32, tag="expb")
            esum = small.tile([TILE, 1], F32, tag="esum")
            nc.scalar.activation(
                out=expb[:], in_=scaled[:], func=mybir.ActivationFunctionType.Exp,
                bias=negmax[:], scale=1.0, accum_out=esum[:],
            )
            rec = small.tile([TILE, 1], F32, tag="rec")
            nc.vector.reciprocal(rec[:], esum[:])

            # unnormalized coefficient matrix Bsel[r, j] = e[j, r%K] when
            # r//K == j%CPG else 0
            bT_ps = psum.tile([K, TILE], F32, tag="bT_ps")
            nc.tensor.transpose(bT_ps[:], expb[:], ident_sb[:])
            bT = small.tile([K, TILE], F32, tag="bT")
            nc.scalar.copy(bT[:], bT_ps[:])
            brep_ps = psum.tile([TILE, TILE], F32, tag="brep_ps")
            nc.tensor.matmul(brep_ps[:], lhsT=i4t_sb[:], rhs=bT[:])
            bsel = med.tile([TILE, TILE], F32, tag="bsel")
            nc.vector.tensor_tensor(
                out=bsel[:], in0=brep_ps[:], in1=m128_sb[:],
                op=mybir.AluOpType.mult,
            )

            # even/odd split by gathered-pair parity: par[r, g] applies to
            # Bsel columns j = 4g..4g+3
            parm = small.tile([TILE, K], F32, tag="parm")
            nc.sync.dma_start(parm[:], par_d[t, :, :])
            parm_ap = parm[:]
            par_bc = bass.AP(
                tensor=parm_ap.tensor,
                offset=parm_ap.offset,
                ap=[parm_ap.ap[0], parm_ap.ap[1], [0, CPG]],
            )
            bselo = med.tile([TILE, TILE], F32, tag="bselo")
            nc.vector.tensor_tensor(
                out=bselo[:].rearrange("p (g c) -> p g c", c=CPG),
                in0=bsel[:].rearrange("p (g c) -> p g c", c=CPG),
                in1=par_bc,
                op=mybir.AluOpType.mult,
            )
            bsele = med.tile([TILE, TILE], F32, tag="bsele")
            nc.vector.tensor_tensor(
                out=bsele[:], in0=bsel[:], in1=bselo[:],
                op=mybir.AluOpType.subtract,
            )

            # weighted neighbor reduction:
            # aggT[f, j] = sum_r xg[r, g(j), par*128 + f] * Bsel[r, j]
            aggT_ps = psum.tile([TILE, TILE], F32, tag="aggT_ps")
            for g in range(K):
                cols = slice(CPG * g, CPG * (g + 1))
                nc.tensor.matmul(
                    aggT_ps[:, cols], lhsT=xg[:, g, 0:FN], rhs=bsele[:, cols],
                    start=True, stop=False,
                )
                nc.tensor.matmul(
                    aggT_ps[:, cols], lhsT=xg[:, g, FN:2 * FN], rhs=bselo[:, cols],
                    start=False, stop=True,
                )
            aggT = med.tile([TILE, TILE], F32, tag="aggT")
            nc.scalar.copy(aggT[:], aggT_ps[:])

            # final projection + softmax normalization:
            # out[n, fo] = (sum_f aggT[f, n] wv[f, fo]) / esum[n]
            out_ps = psum.tile([TILE, FN], F32, tag="out_ps")
            nc.tensor.matmul(out_ps[:], lhsT=aggT[:], rhs=wv_sb[:])
            outs = med.tile([TILE, FN], F32, tag="outs")
            nc.vector.tensor_scalar_mul(outs[:], out_ps[:], rec[:])
            nc.sync.dma_start(out_d[r0:r0 + TILE, :], outs[:])

    nc.finalize()
    return nc


def _host_constants():
    r = np.arange(TILE)
    j = np.arange(TILE)
    m128 = (r[:, None] // K == j[None, :] % CPG).astype(np.float32)
    i4t = (np.arange(TILE)[None, :] % K == np.arange(K)[:, None]).astype(np.float32)
    ident = np.eye(TILE, dtype=np.float32)
    return m128, i4t, ident


def _host_prep(inputs):
    nodes = np.ascontiguousarray(np.asarray(inputs["nodes"], dtype=np.float32))
    nlist = np.asarray(inputs["nlist"]).astype(np.int32)
    edges = np.asarray(inputs["edges"], dtype=np.float32)
    inv_degree = np.asarray(inputs["inv_degree"], dtype=np.float32)
    wq = np.asarray(inputs["wq"], dtype=np.float32)
    wk = np.asarray(inputs["wk"], dtype=np.float32)
    wv = np.asarray(inputs["wv"], dtype=np.float32)

    n_table = nodes.shape[0]
    wqkt = np.ascontiguousarray((wq @ wk.T).astype(np.float32))
    m128, i4t, ident = _host_constants()
    pair_view = np.ascontiguousarray(nodes.reshape(n_table // 2, 2 * FN))

    in_maps = []
    for c in range(NCORES):
        lo = c * SH
        hi = lo + SH

        ed = np.zeros((PAD, K, FE), np.float32)
        ed[:SH] = edges[lo:hi]

        xs = np.zeros((PAD, FN), np.float32)
        xs[:SH] = nodes[lo:hi]
        xselfT = np.ascontiguousarray(xs.reshape(NT, TILE, FN).transpose(0, 2, 1))

        iv = np.ones((PAD, 1), np.float32)
        iv[:SH, 0] = inv_degree[lo:hi]

        nl = np.zeros((PAD, K), np.int32)
        nl[:SH] = nlist[lo:hi]
        # per-tile gather stream: position i holds nlist[t*128 + i//K, i%K]
        streams = nl.reshape(NT, NIDX)
        # wrapped int16 pair indices: idxw[t, p%16, s] = stream[t, s*16+p]
        pidx16 = (streams // 2).astype(np.int16).reshape(NT, NW, 16).transpose(0, 2, 1)
        pidx = np.ascontiguousarray(np.tile(pidx16, (1, 8, 1)))   # [NT, 128, NW]
        # parity par[t, r, g] = stream[t, g*128+r] % 2
        par = np.ascontiguousarray(
            (streams % 2).astype(np.float32).reshape(NT, K, TILE).transpose(0, 2, 1)
        )

        in_maps.append({
            "nodes": pair_view,
            "xselfT": xselfT,
            "edges": ed,
            "pidx": pidx,
            "par": par,
            "invdeg": iv,
            "wv": wv,
            "wqkt": wqkt,
            "m128": m128,
            "i4t": i4t,
            "ident": ident,
        })
    return in_maps


def _run(inputs, trace=False, **kw):
    nc = _CACHE.get("nc")
    if nc is None:
        nc = _build_nc()
        _CACHE["nc"] = nc
    in_maps = _host_prep(inputs)
    res = run_bass_kernel_spmd(
        nc, in_maps, core_ids=list(range(NCORES)), trace=trace, **kw
    )
    out = np.empty((N, FN), np.float32)
    for c in range(NCORES):
        out[c * SH:(c + 1) * SH] = res.results[c]["out"][:SH]
    return out, res


def kernel(**inputs) -> np.ndarray:
    out, _ = _run(inputs, trace=False)
    return out


# revision 9
# speedup vs baseline: 1.1631x; 1.1631x over previous
"""Trainium2 Bass kernel: GNN attention message-passing (AMP layer).

reference math (per node n, K neighbors):
    q      = nodes @ wq                       [N, FE]
    rq     = q @ wk.T = nodes @ (wq @ wk.T)   [N, FE]   (host folds wq@wk.T)
    logit[n,k] = inv_degree[n] * (edges[n,k,:] . rq[n,:])
    b      = softmax_k(logit)
    agg[n] = sum_k b[n,k] * nodes[nlist[n,k]]
    out    = agg @ wv

Distribution: node axis N sharded over 8 cores (6250 rows each, padded to
6272 = 49 tiles of 128). The full nodes table is replicated into every
core's DRAM; the neighbor gather is a per-core dma_gather. No collectives.

The gather uses int16 indices (hardware constraint), which cannot address
50000 rows directly, so the table is viewed as 25000 PAIR tokens of 2x128
floats and idx = nlist//2; the wrong half of each gathered pair is masked
out in the weighted-reduction coefficient matrix (even/odd split).

Per 128-node tile on each core:
  - dma_gather 4096 pair tokens (single_packet=False: >64 desc/engine)
  - logits/softmax on DVE+ACT (per-partition grouped dot products)
  - weighted neighbor reduction as 2x32 small PE matmuls against even/odd
    block-diagonal coefficient matrices built on-chip from softmax output
  - final projection by wv on PE
"""

from contextlib import ExitStack

import ml_dtypes
import numpy as np

import concourse.bass as bass
import concourse.bacc as bacc
import concourse.tile as tile
from concourse import mybir
from concourse.bass_utils import run_bass_kernel_spmd

N, K, FN, FE = 50000, 32, 128, 64
NCORES = 8
SH = N // NCORES            # rows per core (6250)
TILE = 128
NT = -(-SH // TILE)         # tiles per core (49)
PAD = NT * TILE             # padded rows per core (6272)
CPG = TILE // K             # nodes completed per gather block (4)
NIDX = TILE * K             # gathered rows per tile (4096)
NW = NIDX // 16             # wrapped idx columns (256)

F32 = mybir.dt.float32
BF16 = mybir.dt.bfloat16
I16 = mybir.dt.int16

_CACHE: dict = {}


def _build_nc(n_table: int | None = None, nt: int | None = None):
    """Build the SPMD per-core graph. Identical on all 8 cores; only the
    DRAM input contents differ per core."""
    n_table = N if n_table is None else n_table
    nt = NT if nt is None else nt
    pad = nt * TILE
    npair = n_table // 2
    nc = bacc.Bacc()

    nodes_d = nc.dram_tensor("nodes", [npair, 2 * FN], BF16, kind="ExternalInput")
    xselfT_d = nc.dram_tensor("xselfT", [nt, FN, TILE], F32, kind="ExternalInput")
    edges_d = nc.dram_tensor("edges", [pad, K, FE], F32, kind="ExternalInput")
    pidx_d = nc.dram_tensor("pidx", [nt, 128, NW], I16, kind="ExternalInput")
    par_d = nc.dram_tensor("par", [nt, TILE, K], BF16, kind="ExternalInput")
    inv_d = nc.dram_tensor("invdeg", [pad, 1], F32, kind="ExternalInput")
    wv_d = nc.dram_tensor("wv", [FN, FN], BF16, kind="ExternalInput")
    wqkt_d = nc.dram_tensor("wqkt", [FN, FE], F32, kind="ExternalInput")
    m128_d = nc.dram_tensor("m128", [TILE, TILE], F32, kind="ExternalInput")
    i4t_d = nc.dram_tensor("i4t", [K, TILE], BF16, kind="ExternalInput")
    ident_d = nc.dram_tensor("ident", [TILE, TILE], F32, kind="ExternalInput")
    out_d = nc.dram_tensor("out", [pad, FN], F32, kind="ExternalOutput")

    with tile.TileContext(nc) as tc, ExitStack() as ctx:
        consts = ctx.enter_context(tc.tile_pool(name="consts", bufs=1))
        big = ctx.enter_context(tc.tile_pool(name="big", bufs=3))
        gat = ctx.enter_context(tc.tile_pool(name="gat", bufs=3))
        med = ctx.enter_context(tc.tile_pool(name="med", bufs=3))
        small = ctx.enter_context(tc.tile_pool(name="small", bufs=4))
        psum = ctx.enter_context(tc.tile_pool(name="psum", bufs=1, space="PSUM"))

        wv_sb = consts.tile([FN, FN], BF16)
        nc.sync.dma_start(wv_sb[:], wv_d[:, :])
        wqkt_sb = consts.tile([FN, FE], F32)
        nc.sync.dma_start(wqkt_sb[:], wqkt_d[:, :])
        m128_sb = consts.tile([TILE, TILE], F32)
        nc.sync.dma_start(m128_sb[:], m128_d[:, :])
        i4t_sb = consts.tile([K, TILE], BF16)
        nc.sync.dma_start(i4t_sb[:], i4t_d[:, :])
        ident_sb = consts.tile([TILE, TILE], F32)
        nc.sync.dma_start(ident_sb[:], ident_d[:, :])

        for t in range(nt):
            r0 = t * TILE

            # pair-token gather: element i lands at xg[i%128, i//128, :]
            pidx = small.tile([128, NW], I16, tag="pidx")
            nc.sync.dma_start(pidx[:], pidx_d[t, :, :])
            xg = gat.tile([TILE, K, 2 * FN], BF16, tag="xg")
            nc.gpsimd.dma_gather(
                xg[:], nodes_d[:, :], pidx[:],
                num_idxs=NIDX, num_idxs_reg=NIDX, elem_size=2 * FN,
                single_packet=False,
            )

            # self features (pre-transposed on host): xsT[f, n]
            xsT = med.tile([FN, TILE], F32, tag="xsT")
            nc.sync.dma_start(xsT[:], xselfT_d[t, :, :])

            # rq[n, c] = sum_f xself[n, f] * (wq@wk.T)[f, c]
            rq_ps = psum.tile([TILE, FE], F32, tag="rq_ps")
            nc.tensor.matmul(rq_ps[:], lhsT=xsT[:], rhs=wqkt_sb[:])
            rq = small.tile([TILE, FE], F32, tag="rq")
            nc.scalar.copy(rq[:], rq_ps[:])

            # edges tile + logits: dots[n, k] = sum_c edges[n,k,c] * rq[n,c]
            ed = big.tile([TILE, K, FE], F32, tag="ed")
            nc.sync.dma_start(ed[:], edges_d[r0:r0 + TILE, :, :])
            prod = big.tile([TILE, K, FE], F32, tag="prod")
            rq_ap = rq[:]
            rq_bc = bass.AP(
                tensor=rq_ap.tensor,
                offset=rq_ap.offset,
                ap=[rq_ap.ap[0], [0, K], rq_ap.ap[1]],
            )
            nc.vector.tensor_tensor(
                out=prod[:], in0=ed[:], in1=rq_bc, op=mybir.AluOpType.mult
            )
            dots = small.tile([TILE, K], F32, tag="dots")
            nc.vector.tensor_reduce(
                out=dots[:], in_=prod[:], axis=mybir.AxisListType.X,
                op=mybir.AluOpType.add,
            )

            # scale by inv_degree, softmax over k (normalization deferred)
            iv = small.tile([TILE, 1], F32, tag="iv")
            nc.sync.dma_start(iv[:], inv_d[r0:r0 + TILE, :])
            scaled = small.tile([TILE, K], F32, tag="scaled")
            nc.vector.tensor_scalar_mul(scaled[:], dots[:], iv[:])
            negmax = small.tile([TILE, 1], F32, tag="negmax")
            nc.vector.reduce_max(
                out=negmax[:], in_=scaled[:], axis=mybir.AxisListType.X, negate=True
            )
            expb = small.tile([TILE, K], F32, tag="expb")
            esum = small.tile([TILE, 1], F32, tag="esum")
            nc.scalar.activation(
                out=expb[:], in_=scaled[:], func=mybir.ActivationFunctionType.Exp,
                bias=negmax[:], scale=1.0, accum_out=esum[:],
            )
            rec = small.tile([TILE, 1], F32, tag="rec")
            nc.vector.reciprocal(rec[:], esum[:])

            # unnormalized coefficient matrix Bsel[r, j] = e[j, r%K] when
            # r//K == j%CPG else 0
            bT_ps = psum.tile([K, TILE], F32, tag="bT_ps")
            nc.tensor.transpose(bT_ps[:], expb[:], ident_sb[:])
            bT = small.tile([K, TILE], BF16, tag="bT")
            nc.scalar.copy(bT[:], bT_ps[:])
            brep_ps = psum.tile([TILE, TILE], F32, tag="brep_ps")
            nc.tensor.matmul(brep_ps[:], lhsT=i4t_sb[:], rhs=bT[:])
            bsel = med.tile([TILE, TILE], BF16, tag="bsel")
            nc.vector.tensor_tensor(
                out=bsel[:], in0=brep_ps[:], in1=m128_sb[:],
                op=mybir.AluOpType.mult,
            )

            # even/odd split by gathered-pair parity: par[r, g] applies to
            # Bsel columns j = 4g..4g+3
            parm = small.tile([TILE, K], BF16, tag="parm")
            nc.sync.dma_start(parm[:], par_d[t, :, :])
            parm_ap = parm[:]
            par_bc = bass.AP(
                tensor=parm_ap.tensor,
                offset=parm_ap.offset,
                ap=[parm_ap.ap[0], parm_ap.ap[1], [0, CPG]],
            )
            bselo = med.tile([TILE, TILE], BF16, tag="bselo")
            nc.vector.tensor_tensor(
                out=bselo[:].rearrange("p (g c) -> p g c", c=CPG),
                in0=bsel[:].rearrange("p (g c) -> p g c", c=CPG),
                in1=par_bc,
                op=mybir.AluOpType.mult,
            )
            bsele = med.tile([TILE, TILE], BF16, tag="bsele")
            nc.vector.tensor_tensor(
                out=bsele[:], in0=bsel[:], in1=bselo[:],
                op=mybir.AluOpType.subtract,
            )

            # weighted neighbor reduction:
            # aggT[f, j] = sum_r xg[r, g(j), par*128 + f] * Bsel[r, j]
            aggT_ps = psum.tile([TILE, TILE], F32, tag="aggT_ps")
            for g in range(K):
                cols = slice(CPG * g, CPG * (g + 1))
                nc.tensor.matmul(
                    aggT_ps[:, cols], lhsT=xg[:, g, 0:FN], rhs=bsele[:, cols],
                    start=True, stop=False,
                )
                nc.tensor.matmul(
                    aggT_ps[:, cols], lhsT=xg[:, g, FN:2 * FN], rhs=bselo[:, cols],
                    start=False, stop=True,
                )
            aggT = med.tile([TILE, TILE], BF16, tag="aggT")
            nc.scalar.copy(aggT[:], aggT_ps[:])

            # final projection + softmax normalization:
            # out[n, fo] = (sum_f aggT[f, n] wv[f, fo]) / esum[n]
            out_ps = psum.tile([TILE, FN], F32, tag="out_ps")
            nc.tensor.matmul(out_ps[:], lhsT=aggT[:], rhs=wv_sb[:])
            outs = med.tile([TILE, FN], F32, tag="outs")
            nc.vector.tensor_scalar_mul(outs[:], out_ps[:], rec[:])
            nc.sync.dma_start(out_d[r0:r0 + TILE, :], outs[:])

    nc.finalize()
    return nc


def _host_constants():
    r = np.arange(TILE)
    j = np.arange(TILE)
    m128 = (r[:, None] // K == j[None, :] % CPG).astype(np.float32)
    i4t = (np.arange(TILE)[None, :] % K ==
           np.arange(K)[:, None]).astype(ml_dtypes.bfloat16)
    ident = np.eye(TILE, dtype=np.float32)
    return m128, i4t, ident


def _host_prep(inputs):
    nodes = np.ascontiguousarray(np.asarray(inputs["nodes"], dtype=np.float32))
    nlist = np.asarray(inputs["nlist"]).astype(np.int32)
    edges = np.asarray(inputs["edges"], dtype=np.float32)
    inv_degree = np.asarray(inputs["inv_degree"], dtype=np.float32)
    wq = np.asarray(inputs["wq"], dtype=np.float32)
    wk = np.asarray(inputs["wk"], dtype=np.float32)
    wv = np.asarray(inputs["wv"], dtype=np.float32)

    n_table = nodes.shape[0]
    wqkt = np.ascontiguousarray((wq @ wk.T).astype(np.float32))
    m128, i4t, ident = _host_constants()
    pair_view = np.ascontiguousarray(
        nodes.reshape(n_table // 2, 2 * FN).astype(ml_dtypes.bfloat16))

    in_maps = []
    for c in range(NCORES):
        lo = c * SH
        hi = lo + SH

        ed = np.zeros((PAD, K, FE), np.float32)
        ed[:SH] = edges[lo:hi]

        xs = np.zeros((PAD, FN), np.float32)
        xs[:SH] = nodes[lo:hi]
        xselfT = np.ascontiguousarray(xs.reshape(NT, TILE, FN).transpose(0, 2, 1))

        iv = np.ones((PAD, 1), np.float32)
        iv[:SH, 0] = inv_degree[lo:hi]

        nl = np.zeros((PAD, K), np.int32)
        nl[:SH] = nlist[lo:hi]
        # per-tile gather stream: position i holds nlist[t*128 + i//K, i%K]
        streams = nl.reshape(NT, NIDX)
        # wrapped int16 pair indices: idxw[t, p%16, s] = stream[t, s*16+p]
        pidx16 = (streams // 2).astype(np.int16).reshape(NT, NW, 16).transpose(0, 2, 1)
        pidx = np.ascontiguousarray(np.tile(pidx16, (1, 8, 1)))   # [NT, 128, NW]
        # parity par[t, r, g] = stream[t, g*128+r] % 2
        par = np.ascontiguousarray(
            (streams % 2).astype(ml_dtypes.bfloat16)
            .reshape(NT, K, TILE).transpose(0, 2, 1)
        )

        in_maps.append({
            "nodes": pair_view,
            "xselfT": xselfT,
            "edges": ed,
            "pidx": pidx,
            "par": par,
            "invdeg": iv,
            "wv": wv.astype(ml_dtypes.bfloat16),
            "wqkt": wqkt,
            "m128": m128,
            "i4t": i4t,
            "ident": ident,
        })
    return in_maps


def _run(inputs, trace=False, **kw):
    nc = _CACHE.get("nc")
    if nc is None:
        nc = _build_nc()
        _CACHE["nc"] = nc
    in_maps = _host_prep(inputs)
    res = run_bass_kernel_spmd(
        nc, in_maps, core_ids=list(range(NCORES)), trace=trace, **kw
    )
    out = np.empty((N, FN), np.float32)
    for c in range(NCORES):
        out[c * SH:(c + 1) * SH] = res.results[c]["out"][:SH]
    return out, res


def kernel(**inputs) -> np.ndarray:
    out, _ = _run(inputs, trace=False)
    return out


# revision 10
# speedup vs baseline: 1.2225x; 1.0510x over previous
"""Trainium2 Bass kernel: GNN attention message-passing (AMP layer).

reference math (per node n, K neighbors):
    q      = nodes @ wq                       [N, FE]
    rq     = q @ wk.T = nodes @ (wq @ wk.T)   [N, FE]   (host folds wq@wk.T)
    logit[n,k] = inv_degree[n] * (edges[n,k,:] . rq[n,:])
    b      = softmax_k(logit)
    agg[n] = sum_k b[n,k] * nodes[nlist[n,k]]
    out    = agg @ wv

Distribution: node axis N sharded over 8 cores (6250 rows each, padded to
6272 = 49 tiles of 128). The full nodes table is replicated into every
core's DRAM; the neighbor gather is a per-core dma_gather. No collectives.

The gather uses int16 indices (hardware constraint), which cannot address
50000 rows directly, so the table is viewed as 25000 PAIR tokens of 2x128
floats and idx = nlist//2; the wrong half of each gathered pair is masked
out in the weighted-reduction coefficient matrix (even/odd split).

Per 128-node tile on each core:
  - dma_gather 4096 pair tokens (single_packet=False: >64 desc/engine)
  - logits/softmax on DVE+ACT (per-partition grouped dot products)
  - weighted neighbor reduction as 2x32 small PE matmuls against even/odd
    block-diagonal coefficient matrices built on-chip from softmax output
  - final projection by wv on PE
"""

from contextlib import ExitStack

import ml_dtypes
import numpy as np

import concourse.bass as bass
import concourse.bacc as bacc
import concourse.tile as tile
from concourse import mybir
from concourse.bass_utils import run_bass_kernel_spmd

N, K, FN, FE = 50000, 32, 128, 64
NCORES = 8
SH = N // NCORES            # rows per core (6250)
TILE = 128
NT = -(-SH // TILE)         # tiles per core (49)
PAD = NT * TILE             # padded rows per core (6272)
CPG = TILE // K             # nodes completed per gather block (4)
NIDX = TILE * K             # gathered rows per tile (4096)
NW = NIDX // 16             # wrapped idx columns (256)

F32 = mybir.dt.float32
BF16 = mybir.dt.bfloat16
I16 = mybir.dt.int16

_CACHE: dict = {}


def _build_nc(n_table: int | None = None, nt: int | None = None):
    """Build the SPMD per-core graph. Identical on all 8 cores; only the
    DRAM input contents differ per core."""
    n_table = N if n_table is None else n_table
    nt = NT if nt is None else nt
    pad = nt * TILE
    npair = n_table // 2
    nc = bacc.Bacc()

    nodes_d = nc.dram_tensor("nodes", [npair, 2 * FN], BF16, kind="ExternalInput")
    xselfT_d = nc.dram_tensor("xselfT", [nt, FN, TILE], F32, kind="ExternalInput")
    edges_d = nc.dram_tensor("edges", [pad, K, FE], F32, kind="ExternalInput")
    pidx_d = nc.dram_tensor("pidx", [nt, 128, NW], I16, kind="ExternalInput")
    par_d = nc.dram_tensor("par", [nt, TILE, K], BF16, kind="ExternalInput")
    inv_d = nc.dram_tensor("invdeg", [pad, 1], F32, kind="ExternalInput")
    wv_d = nc.dram_tensor("wv", [FN, FN], BF16, kind="ExternalInput")
    wqkt_d = nc.dram_tensor("wqkt", [FN, FE], F32, kind="ExternalInput")
    m128_d = nc.dram_tensor("m128", [TILE, TILE], F32, kind="ExternalInput")
    i4t_d = nc.dram_tensor("i4t", [K, TILE], BF16, kind="ExternalInput")
    ident_d = nc.dram_tensor("ident", [TILE, TILE], F32, kind="ExternalInput")
    out_d = nc.dram_tensor("out", [pad, FN], F32, kind="ExternalOutput")

    with tile.TileContext(nc) as tc, ExitStack() as ctx:
        consts = ctx.enter_context(tc.tile_pool(name="consts", bufs=1))
        big = ctx.enter_context(tc.tile_pool(name="big", bufs=3))
        gat = ctx.enter_context(tc.tile_pool(name="gat", bufs=4))
        idxp = ctx.enter_context(tc.tile_pool(name="idxp", bufs=8))
        med = ctx.enter_context(tc.tile_pool(name="med", bufs=3))
        small = ctx.enter_context(tc.tile_pool(name="small", bufs=4))
        psum = ctx.enter_context(tc.tile_pool(name="psum", bufs=1, space="PSUM"))

        wv_sb = consts.tile([FN, FN], BF16)
        nc.sync.dma_start(wv_sb[:], wv_d[:, :])
        wqkt_sb = consts.tile([FN, FE], F32)
        nc.sync.dma_start(wqkt_sb[:], wqkt_d[:, :])
        m128_sb = consts.tile([TILE, TILE], F32)
        nc.sync.dma_start(m128_sb[:], m128_d[:, :])
        i4t_sb = consts.tile([K, TILE], BF16)
        nc.sync.dma_start(i4t_sb[:], i4t_d[:, :])
        ident_sb = consts.tile([TILE, TILE], F32)
        nc.sync.dma_start(ident_sb[:], ident_d[:, :])

        for t in range(nt):
            r0 = t * TILE

            # pair-token gather: element i lands at xg[i%128, i//128, :]
            pidx = idxp.tile([128, NW], I16, tag="pidx")
            nc.sync.dma_start(pidx[:], pidx_d[t, :, :])
            xg = gat.tile([TILE, K, 2 * FN], BF16, tag="xg")
            nc.gpsimd.dma_gather(
                xg[:], nodes_d[:, :], pidx[:],
                num_idxs=NIDX, num_idxs_reg=NIDX, elem_size=2 * FN,
                single_packet=False,
            )

            # self features (pre-transposed on host): xsT[f, n]
            xsT = med.tile([FN, TILE], F32, tag="xsT")
            nc.sync.dma_start(xsT[:], xselfT_d[t, :, :])

            # rq[n, c] = sum_f xself[n, f] * (wq@wk.T)[f, c]
            rq_ps = psum.tile([TILE, FE], F32, tag="rq_ps")
            nc.tensor.matmul(rq_ps[:], lhsT=xsT[:], rhs=wqkt_sb[:])
            rq = small.tile([TILE, FE], F32, tag="rq")
            nc.scalar.copy(rq[:], rq_ps[:])

            # edges tile + logits: dots[n, k] = sum_c edges[n,k,c] * rq[n,c]
            ed = big.tile([TILE, K, FE], F32, tag="ed")
            nc.sync.dma_start(ed[:], edges_d[r0:r0 + TILE, :, :])
            prod = big.tile([TILE, K, FE], F32, tag="prod")
            rq_ap = rq[:]
            rq_bc = bass.AP(
                tensor=rq_ap.tensor,
                offset=rq_ap.offset,
                ap=[rq_ap.ap[0], [0, K], rq_ap.ap[1]],
            )
            nc.vector.tensor_tensor(
                out=prod[:], in0=ed[:], in1=rq_bc, op=mybir.AluOpType.mult
            )
            dots = small.tile([TILE, K], F32, tag="dots")
            nc.vector.tensor_reduce(
                out=dots[:], in_=prod[:], axis=mybir.AxisListType.X,
                op=mybir.AluOpType.add,
            )

            # scale by inv_degree, softmax over k (normalization deferred)
            iv = small.tile([TILE, 1], F32, tag="iv")
            nc.sync.dma_start(iv[:], inv_d[r0:r0 + TILE, :])
            scaled = small.tile([TILE, K], F32, tag="scaled")
            nc.scalar.mul(scaled[:], dots[:], iv[:])
            negmax = small.tile([TILE, 1], F32, tag="negmax")
            nc.vector.reduce_max(
                out=negmax[:], in_=scaled[:], axis=mybir.AxisListType.X, negate=True
            )
            expb = small.tile([TILE, K], F32, tag="expb")
            esum = small.tile([TILE, 1], F32, tag="esum")
            nc.scalar.activation(
                out=expb[:], in_=scaled[:], func=mybir.ActivationFunctionType.Exp,
                bias=negmax[:], scale=1.0, accum_out=esum[:],
            )
            rec = small.tile([TILE, 1], F32, tag="rec")
            nc.vector.reciprocal(rec[:], esum[:])

            # unnormalized coefficient matrix Bsel[r, j] = e[j, r%K] when
            # r//K == j%CPG else 0
            bT_ps = psum.tile([K, TILE], F32, tag="bT_ps")
            nc.tensor.transpose(bT_ps[:], expb[:], ident_sb[:])
            bT = small.tile([K, TILE], BF16, tag="bT")
            nc.scalar.copy(bT[:], bT_ps[:])
            brep_ps = psum.tile([TILE, TILE], F32, tag="brep_ps")
            nc.tensor.matmul(brep_ps[:], lhsT=i4t_sb[:], rhs=bT[:])
            bsel = med.tile([TILE, TILE], BF16, tag="bsel")
            nc.vector.tensor_tensor(
                out=bsel[:], in0=brep_ps[:], in1=m128_sb[:],
                op=mybir.AluOpType.mult,
            )

            # even/odd split by gathered-pair parity: par[r, g] applies to
            # Bsel columns j = 4g..4g+3
            parm = small.tile([TILE, K], BF16, tag="parm")
            nc.sync.dma_start(parm[:], par_d[t, :, :])
            parm_ap = parm[:]
            par_bc = bass.AP(
                tensor=parm_ap.tensor,
                offset=parm_ap.offset,
                ap=[parm_ap.ap[0], parm_ap.ap[1], [0, CPG]],
            )
            bselo = med.tile([TILE, TILE], BF16, tag="bselo")
            nc.vector.tensor_tensor(
                out=bselo[:].rearrange("p (g c) -> p g c", c=CPG),
                in0=bsel[:].rearrange("p (g c) -> p g c", c=CPG),
                in1=par_bc,
                op=mybir.AluOpType.mult,
            )
            bsele = med.tile([TILE, TILE], BF16, tag="bsele")
            nc.vector.tensor_tensor(
                out=bsele[:], in0=bsel[:], in1=bselo[:],
                op=mybir.AluOpType.subtract,
            )

            # weighted neighbor reduction:
            # aggT[f, j] = sum_r xg[r, g(j), par*128 + f] * Bsel[r, j]
            aggT_ps = psum.tile([TILE, TILE], F32, tag="aggT_ps")
            for g in range(K):
                cols = slice(CPG * g, CPG * (g + 1))
                nc.tensor.matmul(
                    aggT_ps[:, cols], lhsT=xg[:, g, 0:FN], rhs=bsele[:, cols],
                    start=True, stop=False,
                )
                nc.tensor.matmul(
                    aggT_ps[:, cols], lhsT=xg[:, g, FN:2 * FN], rhs=bselo[:, cols],
                    start=False, stop=True,
                )
            aggT = med.tile([TILE, TILE], BF16, tag="aggT")
            nc.scalar.copy(aggT[:], aggT_ps[:])

            # final projection + softmax normalization:
            # out[n, fo] = (sum_f aggT[f, n] wv[f, fo]) / esum[n]
            out_ps = psum.tile([TILE, FN], F32, tag="out_ps")
            nc.tensor.matmul(out_ps[:], lhsT=aggT[:], rhs=wv_sb[:])
            outs = med.tile([TILE, FN], F32, tag="outs")
            nc.scalar.mul(outs[:], out_ps[:], rec[:])
            nc.sync.dma_start(out_d[r0:r0 + TILE, :], outs[:])

    nc.finalize()
    return nc


def _host_constants():
    r = np.arange(TILE)
    j = np.arange(TILE)
    m128 = (r[:, None] // K == j[None, :] % CPG).astype(np.float32)
    i4t = (np.arange(TILE)[None, :] % K ==
           np.arange(K)[:, None]).astype(ml_dtypes.bfloat16)
    ident = np.eye(TILE, dtype=np.float32)
    return m128, i4t, ident


def _host_prep(inputs):
    nodes = np.ascontiguousarray(np.asarray(inputs["nodes"], dtype=np.float32))
    nlist = np.asarray(inputs["nlist"]).astype(np.int32)
    edges = np.asarray(inputs["edges"], dtype=np.float32)
    inv_degree = np.asarray(inputs["inv_degree"], dtype=np.float32)
    wq = np.asarray(inputs["wq"], dtype=np.float32)
    wk = np.asarray(inputs["wk"], dtype=np.float32)
    wv = np.asarray(inputs["wv"], dtype=np.float32)

    n_table = nodes.shape[0]
    wqkt = np.ascontiguousarray((wq @ wk.T).astype(np.float32))
    m128, i4t, ident = _host_constants()
    pair_view = np.ascontiguousarray(
        nodes.reshape(n_table // 2, 2 * FN).astype(ml_dtypes.bfloat16))

    in_maps = []
    for c in range(NCORES):
        lo = c * SH
        hi = lo + SH

        ed = np.zeros((PAD, K, FE), np.float32)
        ed[:SH] = edges[lo:hi]

        xs = np.zeros((PAD, FN), np.float32)
        xs[:SH] = nodes[lo:hi]
        xselfT = np.ascontiguousarray(xs.reshape(NT, TILE, FN).transpose(0, 2, 1))

        iv = np.ones((PAD, 1), np.float32)
        iv[:SH, 0] = inv_degree[lo:hi]

        nl = np.zeros((PAD, K), np.int32)
        nl[:SH] = nlist[lo:hi]
        # per-tile gather stream: position i holds nlist[t*128 + i//K, i%K]
        streams = nl.reshape(NT, NIDX)
        # wrapped int16 pair indices: idxw[t, p%16, s] = stream[t, s*16+p]
        pidx16 = (streams // 2).astype(np.int16).reshape(NT, NW, 16).transpose(0, 2, 1)
        pidx = np.ascontiguousarray(np.tile(pidx16, (1, 8, 1)))   # [NT, 128, NW]
        # parity par[t, r, g] = stream[t, g*128+r] % 2
        par = np.ascontiguousarray(
            (streams % 2).astype(ml_dtypes.bfloat16)
            .reshape(NT, K, TILE).transpose(0, 2, 1)
        )

        in_maps.append({
            "nodes": pair_view,
            "xselfT": xselfT,
            "edges": ed,
            "pidx": pidx,
            "par": par,
            "invdeg": iv,
            "wv": wv.astype(ml_dtypes.bfloat16),
            "wqkt": wqkt,
            "m128": m128,
            "i4t": i4t,
            "ident": ident,
        })
    return in_maps


def _run(inputs, trace=False, **kw):
    nc = _CACHE.get("nc")
    if nc is None:
        nc = _build_nc()
        _CACHE["nc"] = nc
    in_maps = _host_prep(inputs)
    res = run_bass_kernel_spmd(
        nc, in_maps, core_ids=list(range(NCORES)), trace=trace, **kw
    )
    out = np.empty((N, FN), np.float32)
    for c in range(NCORES):
        out[c * SH:(c + 1) * SH] = res.results[c]["out"][:SH]
    return out, res


def kernel(**inputs) -> np.ndarray:
    out, _ = _run(inputs, trace=False)
    return out


# revision 12
# speedup vs baseline: 1.2249x; 1.0020x over previous
"""Trainium2 Bass kernel: GNN attention message-passing (AMP layer).

reference math (per node n, K neighbors):
    q      = nodes @ wq                       [N, FE]
    rq     = q @ wk.T = nodes @ (wq @ wk.T)   [N, FE]   (host folds wq@wk.T)
    logit[n,k] = inv_degree[n] * (edges[n,k,:] . rq[n,:])
    b      = softmax_k(logit)
    agg[n] = sum_k b[n,k] * nodes[nlist[n,k]]
    out    = agg @ wv

Distribution: node axis N sharded over 8 cores (6250 rows each, padded to
6272 = 49 tiles of 128). The full nodes table is replicated into every
core's DRAM; the neighbor gather is a per-core dma_gather. No collectives.

The gather uses int16 indices (hardware constraint), which cannot address
50000 rows directly, so the table is viewed as 25000 PAIR tokens of 2x128
floats and idx = nlist//2; the wrong half of each gathered pair is masked
out in the weighted-reduction coefficient matrix (even/odd split).

Per 128-node tile on each core:
  - dma_gather 4096 pair tokens (single_packet=False: >64 desc/engine)
  - logits/softmax on DVE+ACT (per-partition grouped dot products)
  - weighted neighbor reduction as 2x32 small PE matmuls against even/odd
    block-diagonal coefficient matrices built on-chip from softmax output
  - final projection by wv on PE
"""

from contextlib import ExitStack

import ml_dtypes
import numpy as np

import concourse.bass as bass
import concourse.bacc as bacc
import concourse.tile as tile
from concourse import mybir
from concourse.bass_utils import run_bass_kernel_spmd

N, K, FN, FE = 50000, 32, 128, 64
NCORES = 8
SH = N // NCORES            # rows per core (6250)
TILE = 128
NT = -(-SH // TILE)         # tiles per core (49)
PAD = NT * TILE             # padded rows per core (6272)
CPG = TILE // K             # nodes completed per gather block (4)
NIDX = TILE * K             # gathered rows per tile (4096)
NW = NIDX // 16             # wrapped idx columns (256)

F32 = mybir.dt.float32
BF16 = mybir.dt.bfloat16
I16 = mybir.dt.int16

_CACHE: dict = {}


def _build_nc(n_table: int | None = None, nt: int | None = None):
    """Build the SPMD per-core graph. Identical on all 8 cores; only the
    DRAM input contents differ per core."""
    n_table = N if n_table is None else n_table
    nt = NT if nt is None else nt
    pad = nt * TILE
    npair = n_table // 2
    nc = bacc.Bacc()

    nodes_d = nc.dram_tensor("nodes", [npair, 2 * FN], BF16, kind="ExternalInput")
    xselfT_d = nc.dram_tensor("xselfT", [nt, FN, TILE], F32, kind="ExternalInput")
    edges_d = nc.dram_tensor("edges", [pad, K, FE], F32, kind="ExternalInput")
    pidx_d = nc.dram_tensor("pidx", [nt, 128, NW], I16, kind="ExternalInput")
    par_d = nc.dram_tensor("par", [nt, TILE, K], BF16, kind="ExternalInput")
    inv_d = nc.dram_tensor("invdeg", [pad, 1], F32, kind="ExternalInput")
    wv_d = nc.dram_tensor("wv", [FN, FN], BF16, kind="ExternalInput")
    wqkt_d = nc.dram_tensor("wqkt", [FN, FE], F32, kind="ExternalInput")
    m128_d = nc.dram_tensor("m128", [TILE, TILE], F32, kind="ExternalInput")
    i4t_d = nc.dram_tensor("i4t", [K, TILE], BF16, kind="ExternalInput")
    ident_d = nc.dram_tensor("ident", [TILE, TILE], F32, kind="ExternalInput")
    out_d = nc.dram_tensor("out", [pad, FN], F32, kind="ExternalOutput")

    with tile.TileContext(nc) as tc, ExitStack() as ctx:
        consts = ctx.enter_context(tc.tile_pool(name="consts", bufs=1))
        big = ctx.enter_context(tc.tile_pool(name="big", bufs=3))
        gat = ctx.enter_context(tc.tile_pool(name="gat", bufs=4))
        idxp = ctx.enter_context(tc.tile_pool(name="idxp", bufs=8))
        med = ctx.enter_context(tc.tile_pool(name="med", bufs=3))
        small = ctx.enter_context(tc.tile_pool(name="small", bufs=4))
        psum = ctx.enter_context(tc.tile_pool(name="psum", bufs=1, space="PSUM"))

        wv_sb = consts.tile([FN, FN], BF16)
        nc.sync.dma_start(wv_sb[:], wv_d[:, :])
        wqkt_sb = consts.tile([FN, FE], F32)
        nc.sync.dma_start(wqkt_sb[:], wqkt_d[:, :])
        m128_sb = consts.tile([TILE, TILE], F32)
        nc.sync.dma_start(m128_sb[:], m128_d[:, :])
        i4t_sb = consts.tile([K, TILE], BF16)
        nc.sync.dma_start(i4t_sb[:], i4t_d[:, :])
        ident_sb = consts.tile([TILE, TILE], F32)
        nc.sync.dma_start(ident_sb[:], ident_d[:, :])

        valid_last = SH * K - (nt - 1) * NIDX if nt * TILE == PAD else NIDX
        for t in range(nt):
            r0 = t * TILE

            # pair-token gather: element i lands at xg[i%128, i//128, :];
            # the last tile skips its trailing pad rows (-1 indices)
            pidx = idxp.tile([128, NW], I16, tag="pidx")
            nc.sync.dma_start(pidx[:], pidx_d[t, :, :])
            xg = gat.tile([TILE, K, 2 * FN], BF16, tag="xg")
            reg = NIDX if t < nt - 1 else valid_last
            nc.gpsimd.dma_gather(
                xg[:], nodes_d[:, :], pidx[:],
                num_idxs=NIDX, num_idxs_reg=reg, elem_size=2 * FN,
                single_packet=False,
            )

            # self features (pre-transposed on host): xsT[f, n]
            xsT = med.tile([FN, TILE], F32, tag="xsT")
            nc.sync.dma_start(xsT[:], xselfT_d[t, :, :])

            # rq[n, c] = sum_f xself[n, f] * (wq@wk.T)[f, c]
            rq_ps = psum.tile([TILE, FE], F32, tag="rq_ps")
            nc.tensor.matmul(rq_ps[:], lhsT=xsT[:], rhs=wqkt_sb[:])
            rq = small.tile([TILE, FE], F32, tag="rq")
            nc.scalar.copy(rq[:], rq_ps[:])

            # edges tile + logits: dots[n, k] = sum_c edges[n,k,c] * rq[n,c]
            ed = big.tile([TILE, K, FE], F32, tag="ed")
            nc.sync.dma_start(ed[:], edges_d[r0:r0 + TILE, :, :])
            prod = big.tile([TILE, K, FE], F32, tag="prod")
            rq_ap = rq[:]
            rq_bc = bass.AP(
                tensor=rq_ap.tensor,
                offset=rq_ap.offset,
                ap=[rq_ap.ap[0], [0, K], rq_ap.ap[1]],
            )
            nc.vector.tensor_tensor(
                out=prod[:], in0=ed[:], in1=rq_bc, op=mybir.AluOpType.mult
            )
            dots = small.tile([TILE, K], F32, tag="dots")
            nc.vector.tensor_reduce(
                out=dots[:], in_=prod[:], axis=mybir.AxisListType.X,
                op=mybir.AluOpType.add,
            )

            # scale by inv_degree, softmax over k (normalization deferred)
            iv = small.tile([TILE, 1], F32, tag="iv")
            nc.sync.dma_start(iv[:], inv_d[r0:r0 + TILE, :])
            scaled = small.tile([TILE, K], F32, tag="scaled")
            nc.scalar.mul(scaled[:], dots[:], iv[:])
            negmax = small.tile([TILE, 1], F32, tag="negmax")
            nc.vector.reduce_max(
                out=negmax[:], in_=scaled[:], axis=mybir.AxisListType.X, negate=True
            )
            expb = small.tile([TILE, K], F32, tag="expb")
            esum = small.tile([TILE, 1], F32, tag="esum")
            nc.scalar.activation(
                out=expb[:], in_=scaled[:], func=mybir.ActivationFunctionType.Exp,
                bias=negmax[:], scale=1.0, accum_out=esum[:],
            )
            rec = small.tile([TILE, 1], F32, tag="rec")
            nc.vector.reciprocal(rec[:], esum[:])

            # unnormalized coefficient matrix Bsel[r, j] = e[j, r%K] when
            # r//K == j%CPG else 0
            bT_ps = psum.tile([K, TILE], F32, tag="bT_ps")
            nc.tensor.transpose(bT_ps[:], expb[:], ident_sb[:])
            bT = small.tile([K, TILE], BF16, tag="bT")
            nc.scalar.copy(bT[:], bT_ps[:])
            brep_ps = psum.tile([TILE, TILE], F32, tag="brep_ps")
            nc.tensor.matmul(brep_ps[:], lhsT=i4t_sb[:], rhs=bT[:])
            bsel = med.tile([TILE, TILE], BF16, tag="bsel")
            nc.vector.tensor_tensor(
                out=bsel[:], in0=brep_ps[:], in1=m128_sb[:],
                op=mybir.AluOpType.mult,
            )

            # even/odd split by gathered-pair parity: par[r, g] applies to
            # Bsel columns j = 4g..4g+3
            parm = small.tile([TILE, K], BF16, tag="parm")
            nc.sync.dma_start(parm[:], par_d[t, :, :])
            parm_ap = parm[:]
            par_bc = bass.AP(
                tensor=parm_ap.tensor,
                offset=parm_ap.offset,
                ap=[parm_ap.ap[0], parm_ap.ap[1], [0, CPG]],
            )
            bselo = med.tile([TILE, TILE], BF16, tag="bselo")
            nc.vector.tensor_tensor(
                out=bselo[:].rearrange("p (g c) -> p g c", c=CPG),
                in0=bsel[:].rearrange("p (g c) -> p g c", c=CPG),
                in1=par_bc,
                op=mybir.AluOpType.mult,
            )
            bsele = med.tile([TILE, TILE], BF16, tag="bsele")
            nc.vector.tensor_tensor(
                out=bsele[:], in0=bsel[:], in1=bselo[:],
                op=mybir.AluOpType.subtract,
            )

            # weighted neighbor reduction:
            # aggT[f, j] = sum_r xg[r, g(j), par*128 + f] * Bsel[r, j]
            aggT_ps = psum.tile([TILE, TILE], F32, tag="aggT_ps")
            for g in range(K):
                cols = slice(CPG * g, CPG * (g + 1))
                nc.tensor.matmul(
                    aggT_ps[:, cols], lhsT=xg[:, g, 0:FN], rhs=bsele[:, cols],
                    start=True, stop=False,
                )
                nc.tensor.matmul(
                    aggT_ps[:, cols], lhsT=xg[:, g, FN:2 * FN], rhs=bselo[:, cols],
                    start=False, stop=True,
                )
            aggT = med.tile([TILE, TILE], BF16, tag="aggT")
            nc.scalar.copy(aggT[:], aggT_ps[:])

            # final projection + softmax normalization:
            # out[n, fo] = (sum_f aggT[f, n] wv[f, fo]) / esum[n]
            out_ps = psum.tile([TILE, FN], F32, tag="out_ps")
            nc.tensor.matmul(out_ps[:], lhsT=aggT[:], rhs=wv_sb[:])
            outs = med.tile([TILE, FN], F32, tag="outs")
            nc.scalar.mul(outs[:], out_ps[:], rec[:])
            nc.sync.dma_start(out_d[r0:r0 + TILE, :], outs[:])

    nc.finalize()
    return nc


def _host_constants():
    r = np.arange(TILE)
    j = np.arange(TILE)
    m128 = (r[:, None] // K == j[None, :] % CPG).astype(np.float32)
    i4t = (np.arange(TILE)[None, :] % K ==
           np.arange(K)[:, None]).astype(ml_dtypes.bfloat16)
    ident = np.eye(TILE, dtype=np.float32)
    return m128, i4t, ident


def _host_prep(inputs):
    nodes = np.ascontiguousarray(np.asarray(inputs["nodes"], dtype=np.float32))
    nlist = np.asarray(inputs["nlist"]).astype(np.int32)
    edges = np.asarray(inputs["edges"], dtype=np.float32)
    inv_degree = np.asarray(inputs["inv_degree"], dtype=np.float32)
    wq = np.asarray(inputs["wq"], dtype=np.float32)
    wk = np.asarray(inputs["wk"], dtype=np.float32)
    wv = np.asarray(inputs["wv"], dtype=np.float32)

    n_table = nodes.shape[0]
    wqkt = np.ascontiguousarray((wq @ wk.T).astype(np.float32))
    m128, i4t, ident = _host_constants()
    pair_view = np.ascontiguousarray(
        nodes.reshape(n_table // 2, 2 * FN).astype(ml_dtypes.bfloat16))

    in_maps = []
    for c in range(NCORES):
        lo = c * SH
        hi = lo + SH

        ed = np.zeros((PAD, K, FE), np.float32)
        ed[:SH] = edges[lo:hi]

        xs = np.zeros((PAD, FN), np.float32)
        xs[:SH] = nodes[lo:hi]
        xselfT = np.ascontiguousarray(xs.reshape(NT, TILE, FN).transpose(0, 2, 1))

        iv = np.ones((PAD, 1), np.float32)
        iv[:SH, 0] = inv_degree[lo:hi]

        nl = np.zeros((PAD, K), np.int32)
        nl[:SH] = nlist[lo:hi]
        # per-tile gather stream: position i holds nlist[t*128 + i//K, i%K];
        # pad rows on the last tile are marked -1 (descriptor-skipped)
        streams = nl.reshape(NT, NIDX).copy()
        streams[NT - 1, (SH - (NT - 1) * TILE) * K:] = -1
        # wrapped int16 pair indices: idxw[t, p%16, s] = stream[t, s*16+p]
        pidx16 = (streams // 2).astype(np.int16).reshape(NT, NW, 16).transpose(0, 2, 1)
        pidx = np.ascontiguousarray(np.tile(pidx16, (1, 8, 1)))   # [NT, 128, NW]
        # parity par[t, r, g] = stream[t, g*128+r] % 2
        par = np.ascontiguousarray(
            (streams % 2).astype(ml_dtypes.bfloat16)
            .reshape(NT, K, TILE).transpose(0, 2, 1)
        )

        in_maps.append({
            "nodes": pair_view,
            "xselfT": xselfT,
            "edges": ed,
            "pidx": pidx,
            "par": par,
            "invdeg": iv,
            "wv": wv.astype(ml_dtypes.bfloat16),
            "wqkt": wqkt,
            "m128": m128,
            "i4t": i4t,
            "ident": ident,
        })
    return in_maps


def _run(inputs, trace=False, **kw):
    nc = _CACHE.get("nc")
    if nc is None:
        nc = _build_nc()
        _CACHE["nc"] = nc
    in_maps = _host_prep(inputs)
    res = run_bass_kernel_spmd(
        nc, in_maps, core_ids=list(range(NCORES)), trace=trace, **kw
    )
    out = np.empty((N, FN), np.float32)
    for c in range(NCORES):
        out[c * SH:(c + 1) * SH] = res.results[c]["out"][:SH]
    return out, res


def kernel(**inputs) -> np.ndarray:
    out, _ = _run(inputs, trace=False)
    return out


# revision 13
# speedup vs baseline: 1.9619x; 1.6017x over previous
"""Trainium2 Bass kernel: GNN attention message-passing (AMP layer).

reference math (per node n, K neighbors):
    q      = nodes @ wq                       [N, FE]
    rq     = q @ wk.T = nodes @ (wq @ wk.T)   [N, FE]   (host folds wq@wk.T)
    logit[n,k] = inv_degree[n] * (edges[n,k,:] . rq[n,:])
    b      = softmax_k(logit)
    agg[n] = sum_k b[n,k] * nodes[nlist[n,k]]
    out    = agg @ wv

Distribution: node axis N sharded over 8 cores (6250 rows each, padded to
6272 = 49 tiles of 128). The full nodes table is replicated into every
core's DRAM; the neighbor gather is a per-core dma_gather. No collectives.

The gather uses int16 indices (hardware constraint), which cannot address
50000 rows directly, so the table is viewed as 25000 PAIR tokens of 2x128
floats and idx = nlist//2; the wrong half of each gathered pair is masked
out in the weighted-reduction coefficient matrix (even/odd split).

Per 128-node tile on each core:
  - dma_gather 4096 pair tokens (single_packet=False: >64 desc/engine)
  - logits/softmax on DVE+ACT (per-partition grouped dot products)
  - weighted neighbor reduction as 2x32 small PE matmuls against even/odd
    block-diagonal coefficient matrices built on-chip from softmax output
  - final projection by wv on PE
"""

from contextlib import ExitStack

import ml_dtypes
import numpy as np

import concourse.bass as bass
import concourse.bacc as bacc
import concourse.tile as tile
from concourse import mybir
from concourse.bass_utils import run_bass_kernel_spmd

N, K, FN, FE = 50000, 32, 128, 64
NCORES = 8
SH = N // NCORES            # rows per core (6250)
TILE = 128
NT = -(-SH // TILE)         # tiles per core (49)
PAD = NT * TILE             # padded rows per core (6272)
CPG = TILE // K             # nodes completed per gather block (4)
NIDX = TILE * K             # gathered rows per tile (4096)
NW = NIDX // 16             # wrapped idx columns (256)

F32 = mybir.dt.float32
BF16 = mybir.dt.bfloat16
I16 = mybir.dt.int16

_CACHE: dict = {}


def _build_nc(n_table: int | None = None, nt: int | None = None):
    """Build the SPMD per-core graph. Identical on all 8 cores; only the
    DRAM input contents differ per core."""
    n_table = N if n_table is None else n_table
    nt = NT if nt is None else nt
    pad = nt * TILE
    npair = n_table // 2
    nc = bacc.Bacc(num_swdge_queues=2)

    nodes_d = nc.dram_tensor("nodes", [npair, 2 * FN], BF16, kind="ExternalInput")
    xselfT_d = nc.dram_tensor("xselfT", [nt, FN, TILE], F32, kind="ExternalInput")
    edges_d = nc.dram_tensor("edges", [pad, K, FE], F32, kind="ExternalInput")
    pidx_d = nc.dram_tensor("pidx", [nt, 128, NW], I16, kind="ExternalInput")
    par_d = nc.dram_tensor("par", [nt, TILE, K], BF16, kind="ExternalInput")
    inv_d = nc.dram_tensor("invdeg", [pad, 1], F32, kind="ExternalInput")
    wv_d = nc.dram_tensor("wv", [FN, FN], BF16, kind="ExternalInput")
    wqkt_d = nc.dram_tensor("wqkt", [FN, FE], F32, kind="ExternalInput")
    m128_d = nc.dram_tensor("m128", [TILE, TILE], F32, kind="ExternalInput")
    i4t_d = nc.dram_tensor("i4t", [K, TILE], BF16, kind="ExternalInput")
    ident_d = nc.dram_tensor("ident", [TILE, TILE], F32, kind="ExternalInput")
    out_d = nc.dram_tensor("out", [pad, FN], F32, kind="ExternalOutput")

    with tile.TileContext(nc) as tc, ExitStack() as ctx:
        consts = ctx.enter_context(tc.tile_pool(name="consts", bufs=1))
        big = ctx.enter_context(tc.tile_pool(name="big", bufs=3))
        gat = ctx.enter_context(tc.tile_pool(name="gat", bufs=4))
        idxp = ctx.enter_context(tc.tile_pool(name="idxp", bufs=8))
        med = ctx.enter_context(tc.tile_pool(name="med", bufs=3))
        small = ctx.enter_context(tc.tile_pool(name="small", bufs=4))
        psum = ctx.enter_context(tc.tile_pool(name="psum", bufs=1, space="PSUM"))

        wv_sb = consts.tile([FN, FN], BF16)
        nc.sync.dma_start(wv_sb[:], wv_d[:, :])
        wqkt_sb = consts.tile([FN, FE], F32)
        nc.sync.dma_start(wqkt_sb[:], wqkt_d[:, :])
        m128_sb = consts.tile([TILE, TILE], F32)
        nc.sync.dma_start(m128_sb[:], m128_d[:, :])
        i4t_sb = consts.tile([K, TILE], BF16)
        nc.sync.dma_start(i4t_sb[:], i4t_d[:, :])
        ident_sb = consts.tile([TILE, TILE], F32)
        nc.sync.dma_start(ident_sb[:], ident_d[:, :])

        valid_last = SH * K - (nt - 1) * NIDX if nt * TILE == PAD else NIDX
        for t in range(nt):
            r0 = t * TILE

            # pair-token gather: element i lands at xg[i%128, i//128, :];
            # the last tile skips its trailing pad rows (-1 indices)
            pidx = idxp.tile([128, NW], I16, tag="pidx")
            nc.sync.dma_start(pidx[:], pidx_d[t, :, :])
            xg = gat.tile([TILE, K, 2 * FN], BF16, tag="xg")
            reg = NIDX if t < nt - 1 else valid_last
            nc.gpsimd.dma_gather(
                xg[:], nodes_d[:, :], pidx[:],
                num_idxs=NIDX, num_idxs_reg=reg, elem_size=2 * FN,
                single_packet=False, queue_num=t % 2,
            )

            # self features (pre-transposed on host): xsT[f, n]
            xsT = med.tile([FN, TILE], F32, tag="xsT")
            nc.sync.dma_start(xsT[:], xselfT_d[t, :, :])

            # rq[n, c] = sum_f xself[n, f] * (wq@wk.T)[f, c]
            rq_ps = psum.tile([TILE, FE], F32, tag="rq_ps")
            nc.tensor.matmul(rq_ps[:], lhsT=xsT[:], rhs=wqkt_sb[:])
            rq = small.tile([TILE, FE], F32, tag="rq")
            nc.scalar.copy(rq[:], rq_ps[:])

            # edges tile + logits: dots[n, k] = sum_c edges[n,k,c] * rq[n,c]
            ed = big.tile([TILE, K, FE], F32, tag="ed")
            nc.sync.dma_start(ed[:], edges_d[r0:r0 + TILE, :, :])
            prod = big.tile([TILE, K, FE], F32, tag="prod")
            rq_ap = rq[:]
            rq_bc = bass.AP(
                tensor=rq_ap.tensor,
                offset=rq_ap.offset,
                ap=[rq_ap.ap[0], [0, K], rq_ap.ap[1]],
            )
            nc.vector.tensor_tensor(
                out=prod[:], in0=ed[:], in1=rq_bc, op=mybir.AluOpType.mult
            )
            dots = small.tile([TILE, K], F32, tag="dots")
            nc.vector.tensor_reduce(
                out=dots[:], in_=prod[:], axis=mybir.AxisListType.X,
                op=mybir.AluOpType.add,
            )

            # scale by inv_degree, softmax over k (normalization deferred)
            iv = small.tile([TILE, 1], F32, tag="iv")
            nc.sync.dma_start(iv[:], inv_d[r0:r0 + TILE, :])
            scaled = small.tile([TILE, K], F32, tag="scaled")
            nc.scalar.mul(scaled[:], dots[:], iv[:])
            negmax = small.tile([TILE, 1], F32, tag="negmax")
            nc.vector.reduce_max(
                out=negmax[:], in_=scaled[:], axis=mybir.AxisListType.X, negate=True
            )
            expb = small.tile([TILE, K], F32, tag="expb")
            esum = small.tile([TILE, 1], F32, tag="esum")
            nc.scalar.activation(
                out=expb[:], in_=scaled[:], func=mybir.ActivationFunctionType.Exp,
                bias=negmax[:], scale=1.0, accum_out=esum[:],
            )
            rec = small.tile([TILE, 1], F32, tag="rec")
            nc.vector.reciprocal(rec[:], esum[:])

            # unnormalized coefficient matrix Bsel[r, j] = e[j, r%K] when
            # r//K == j%CPG else 0
            bT_ps = psum.tile([K, TILE], F32, tag="bT_ps")
            nc.tensor.transpose(bT_ps[:], expb[:], ident_sb[:])
            bT = small.tile([K, TILE], BF16, tag="bT")
            nc.scalar.copy(bT[:], bT_ps[:])
            brep_ps = psum.tile([TILE, TILE], F32, tag="brep_ps")
            nc.tensor.matmul(brep_ps[:], lhsT=i4t_sb[:], rhs=bT[:])
            bsel = med.tile([TILE, TILE], BF16, tag="bsel")
            nc.vector.tensor_tensor(
                out=bsel[:], in0=brep_ps[:], in1=m128_sb[:],
                op=mybir.AluOpType.mult,
            )

            # even/odd split by gathered-pair parity: par[r, g] applies to
            # Bsel columns j = 4g..4g+3
            parm = small.tile([TILE, K], BF16, tag="parm")
            nc.sync.dma_start(parm[:], par_d[t, :, :])
            parm_ap = parm[:]
            par_bc = bass.AP(
                tensor=parm_ap.tensor,
                offset=parm_ap.offset,
                ap=[parm_ap.ap[0], parm_ap.ap[1], [0, CPG]],
            )
            bselo = med.tile([TILE, TILE], BF16, tag="bselo")
            nc.vector.tensor_tensor(
                out=bselo[:].rearrange("p (g c) -> p g c", c=CPG),
                in0=bsel[:].rearrange("p (g c) -> p g c", c=CPG),
                in1=par_bc,
                op=mybir.AluOpType.mult,
            )
            bsele = med.tile([TILE, TILE], BF16, tag="bsele")
            nc.vector.tensor_tensor(
                out=bsele[:], in0=bsel[:], in1=bselo[:],
                op=mybir.AluOpType.subtract,
            )

            # weighted neighbor reduction:
            # aggT[f, j] = sum_r xg[r, g(j), par*128 + f] * Bsel[r, j]
            aggT_ps = psum.tile([TILE, TILE], F32, tag="aggT_ps")
            for g in range(K):
                cols = slice(CPG * g, CPG * (g + 1))
                nc.tensor.matmul(
                    aggT_ps[:, cols], lhsT=xg[:, g, 0:FN], rhs=bsele[:, cols],
                    start=True, stop=False,
                )
                nc.tensor.matmul(
                    aggT_ps[:, cols], lhsT=xg[:, g, FN:2 * FN], rhs=bselo[:, cols],
                    start=False, stop=True,
                )
            aggT = med.tile([TILE, TILE], BF16, tag="aggT")
            nc.scalar.copy(aggT[:], aggT_ps[:])

            # final projection + softmax normalization:
            # out[n, fo] = (sum_f aggT[f, n] wv[f, fo]) / esum[n]
            out_ps = psum.tile([TILE, FN], F32, tag="out_ps")
            nc.tensor.matmul(out_ps[:], lhsT=aggT[:], rhs=wv_sb[:])
            outs = med.tile([TILE, FN], F32, tag="outs")
            nc.scalar.mul(outs[:], out_ps[:], rec[:])
            nc.sync.dma_start(out_d[r0:r0 + TILE, :], outs[:])

    nc.finalize()
    return nc


def _host_constants():
    r = np.arange(TILE)
    j = np.arange(TILE)
    m128 = (r[:, None] // K == j[None, :] % CPG).astype(np.float32)
    i4t = (np.arange(TILE)[None, :] % K ==
           np.arange(K)[:, None]).astype(ml_dtypes.bfloat16)
    ident = np.eye(TILE, dtype=np.float32)
    return m128, i4t, ident


def _host_prep(inputs):
    nodes = np.ascontiguousarray(np.asarray(inputs["nodes"], dtype=np.float32))
    nlist = np.asarray(inputs["nlist"]).astype(np.int32)
    edges = np.asarray(inputs["edges"], dtype=np.float32)
    inv_degree = np.asarray(inputs["inv_degree"], dtype=np.float32)
    wq = np.asarray(inputs["wq"], dtype=np.float32)
    wk = np.asarray(inputs["wk"], dtype=np.float32)
    wv = np.asarray(inputs["wv"], dtype=np.float32)

    n_table = nodes.shape[0]
    wqkt = np.ascontiguousarray((wq @ wk.T).astype(np.float32))
    m128, i4t, ident = _host_constants()
    pair_view = np.ascontiguousarray(
        nodes.reshape(n_table // 2, 2 * FN).astype(ml_dtypes.bfloat16))

    in_maps = []
    for c in range(NCORES):
        lo = c * SH
        hi = lo + SH

        ed = np.zeros((PAD, K, FE), np.float32)
        ed[:SH] = edges[lo:hi]

        xs = np.zeros((PAD, FN), np.float32)
        xs[:SH] = nodes[lo:hi]
        xselfT = np.ascontiguousarray(xs.reshape(NT, TILE, FN).transpose(0, 2, 1))

        iv = np.ones((PAD, 1), np.float32)
        iv[:SH, 0] = inv_degree[lo:hi]

        nl = np.zeros((PAD, K), np.int32)
        nl[:SH] = nlist[lo:hi]
        # per-tile gather stream: position i holds nlist[t*128 + i//K, i%K];
        # pad rows on the last tile are marked -1 (descriptor-skipped)
        streams = nl.reshape(NT, NIDX).copy()
        streams[NT - 1, (SH - (NT - 1) * TILE) * K:] = -1
        # wrapped int16 pair indices: idxw[t, p%16, s] = stream[t, s*16+p]
        pidx16 = (streams // 2).astype(np.int16).reshape(NT, NW, 16).transpose(0, 2, 1)
        pidx = np.ascontiguousarray(np.tile(pidx16, (1, 8, 1)))   # [NT, 128, NW]
        # parity par[t, r, g] = stream[t, g*128+r] % 2
        par = np.ascontiguousarray(
            (streams % 2).astype(ml_dtypes.bfloat16)
            .reshape(NT, K, TILE).transpose(0, 2, 1)
        )

        in_maps.append({
            "nodes": pair_view,
            "xselfT": xselfT,
            "edges": ed,
            "pidx": pidx,
            "par": par,
            "invdeg": iv,
            "wv": wv.astype(ml_dtypes.bfloat16),
            "wqkt": wqkt,
            "m128": m128,
            "i4t": i4t,
            "ident": ident,
        })
    return in_maps


def _run(inputs, trace=False, **kw):
    nc = _CACHE.get("nc")
    if nc is None:
        nc = _build_nc()
        _CACHE["nc"] = nc
    in_maps = _host_prep(inputs)
    res = run_bass_kernel_spmd(
        nc, in_maps, core_ids=list(range(NCORES)), trace=trace, **kw
    )
    out = np.empty((N, FN), np.float32)
    for c in range(NCORES):
        out[c * SH:(c + 1) * SH] = res.results[c]["out"][:SH]
    return out, res


def kernel(**inputs) -> np.ndarray:
    out, _ = _run(inputs, trace=False)
    return out


# revision 14
# speedup vs baseline: 2.7602x; 1.4069x over previous
"""Trainium2 Bass kernel: GNN attention message-passing (AMP layer).

reference math (per node n, K neighbors):
    q      = nodes @ wq                       [N, FE]
    rq     = q @ wk.T = nodes @ (wq @ wk.T)   [N, FE]   (host folds wq@wk.T)
    logit[n,k] = inv_degree[n] * (edges[n,k,:] . rq[n,:])
    b      = softmax_k(logit)
    agg[n] = sum_k b[n,k] * nodes[nlist[n,k]]
    out    = agg @ wv

Distribution: node axis N sharded over 8 cores (6250 rows each, padded to
6272 = 49 tiles of 128). The full nodes table is replicated into every
core's DRAM; the neighbor gather is a per-core dma_gather. No collectives.

The gather uses int16 indices (hardware constraint), which cannot address
50000 rows directly, so the table is viewed as 25000 PAIR tokens of 2x128
floats and idx = nlist//2; the wrong half of each gathered pair is masked
out in the weighted-reduction coefficient matrix (even/odd split).

Per 128-node tile on each core:
  - dma_gather 4096 pair tokens (single_packet=False: >64 desc/engine)
  - logits/softmax on DVE+ACT (per-partition grouped dot products)
  - weighted neighbor reduction as 2x32 small PE matmuls against even/odd
    block-diagonal coefficient matrices built on-chip from softmax output
  - final projection by wv on PE
"""

from contextlib import ExitStack

import ml_dtypes
import numpy as np

import concourse.bass as bass
import concourse.bacc as bacc
import concourse.tile as tile
from concourse import mybir
from concourse.bass_utils import run_bass_kernel_spmd

N, K, FN, FE = 50000, 32, 128, 64
NCORES = 8
SH = N // NCORES            # rows per core (6250)
TILE = 128
NT = -(-SH // TILE)         # tiles per core (49)
PAD = NT * TILE             # padded rows per core (6272)
CPG = TILE // K             # nodes completed per gather block (4)
NIDX = TILE * K             # gathered rows per tile (4096)
NW = NIDX // 16             # wrapped idx columns (256)

F32 = mybir.dt.float32
BF16 = mybir.dt.bfloat16
I16 = mybir.dt.int16

_CACHE: dict = {}


def _build_nc(n_table: int | None = None, nt: int | None = None):
    """Build the SPMD per-core graph. Identical on all 8 cores; only the
    DRAM input contents differ per core."""
    n_table = N if n_table is None else n_table
    nt = NT if nt is None else nt
    pad = nt * TILE
    npair = n_table // 2
    nc = bacc.Bacc(num_swdge_queues=4)

    nodes_d = nc.dram_tensor("nodes", [npair, 2 * FN], BF16, kind="ExternalInput")
    xselfT_d = nc.dram_tensor("xselfT", [nt, FN, TILE], BF16, kind="ExternalInput")
    edges_d = nc.dram_tensor("edges", [pad, K, FE], BF16, kind="ExternalInput")
    pidx_d = nc.dram_tensor("pidx", [nt, 128, NW], I16, kind="ExternalInput")
    par_d = nc.dram_tensor("par", [nt, TILE, K], BF16, kind="ExternalInput")
    inv_d = nc.dram_tensor("invdeg", [pad, 1], F32, kind="ExternalInput")
    wv_d = nc.dram_tensor("wv", [FN, FN], BF16, kind="ExternalInput")
    wqkt_d = nc.dram_tensor("wqkt", [FN, FE], BF16, kind="ExternalInput")
    m128_d = nc.dram_tensor("m128", [TILE, TILE], F32, kind="ExternalInput")
    i4t_d = nc.dram_tensor("i4t", [K, TILE], BF16, kind="ExternalInput")
    ident_d = nc.dram_tensor("ident", [TILE, TILE], F32, kind="ExternalInput")
    out_d = nc.dram_tensor("out", [pad, FN], F32, kind="ExternalOutput")

    with tile.TileContext(nc) as tc, ExitStack() as ctx:
        consts = ctx.enter_context(tc.tile_pool(name="consts", bufs=1))
        big = ctx.enter_context(tc.tile_pool(name="big", bufs=3))
        gat = ctx.enter_context(tc.tile_pool(name="gat", bufs=4))
        idxp = ctx.enter_context(tc.tile_pool(name="idxp", bufs=8))
        med = ctx.enter_context(tc.tile_pool(name="med", bufs=3))
        small = ctx.enter_context(tc.tile_pool(name="small", bufs=4))
        psum = ctx.enter_context(tc.tile_pool(name="psum", bufs=1, space="PSUM"))

        wv_sb = consts.tile([FN, FN], BF16)
        nc.sync.dma_start(wv_sb[:], wv_d[:, :])
        wqkt_sb = consts.tile([FN, FE], BF16)
        nc.sync.dma_start(wqkt_sb[:], wqkt_d[:, :])
        m128_sb = consts.tile([TILE, TILE], F32)
        nc.sync.dma_start(m128_sb[:], m128_d[:, :])
        i4t_sb = consts.tile([K, TILE], BF16)
        nc.sync.dma_start(i4t_sb[:], i4t_d[:, :])
        ident_sb = consts.tile([TILE, TILE], F32)
        nc.sync.dma_start(ident_sb[:], ident_d[:, :])

        valid_last = SH * K - (nt - 1) * NIDX if nt * TILE == PAD else NIDX
        for t in range(nt):
            r0 = t * TILE

            # pair-token gather: element i lands at xg[i%128, i//128, :];
            # the last tile skips its trailing pad rows (-1 indices)
            pidx = idxp.tile([128, NW], I16, tag="pidx")
            nc.sync.dma_start(pidx[:], pidx_d[t, :, :])
            xg = gat.tile([TILE, K, 2 * FN], BF16, tag="xg")
            reg = NIDX if t < nt - 1 else valid_last
            nc.gpsimd.dma_gather(
                xg[:], nodes_d[:, :], pidx[:],
                num_idxs=NIDX, num_idxs_reg=reg, elem_size=2 * FN,
                single_packet=False, queue_num=t % 4,
            )

            # self features (pre-transposed on host): xsT[f, n]
            xsT = med.tile([FN, TILE], BF16, tag="xsT")
            nc.sync.dma_start(xsT[:], xselfT_d[t, :, :])

            # rq[n, c] = sum_f xself[n, f] * (wq@wk.T)[f, c]
            rq_ps = psum.tile([TILE, FE], F32, tag="rq_ps")
            nc.tensor.matmul(rq_ps[:], lhsT=xsT[:], rhs=wqkt_sb[:])
            rq = small.tile([TILE, FE], F32, tag="rq")
            nc.scalar.copy(rq[:], rq_ps[:])

            # edges tile + logits: dots[n, k] = sum_c edges[n,k,c] * rq[n,c]
            ed = big.tile([TILE, K, FE], BF16, tag="ed")
            nc.sync.dma_start(ed[:], edges_d[r0:r0 + TILE, :, :])
            prod = big.tile([TILE, K, FE], F32, tag="prod")
            rq_ap = rq[:]
            rq_bc = bass.AP(
                tensor=rq_ap.tensor,
                offset=rq_ap.offset,
                ap=[rq_ap.ap[0], [0, K], rq_ap.ap[1]],
            )
            nc.vector.tensor_tensor(
                out=prod[:], in0=ed[:], in1=rq_bc, op=mybir.AluOpType.mult
            )
            dots = small.tile([TILE, K], F32, tag="dots")
            nc.vector.tensor_reduce(
                out=dots[:], in_=prod[:], axis=mybir.AxisListType.X,
                op=mybir.AluOpType.add,
            )

            # scale by inv_degree, softmax over k (normalization deferred)
            iv = small.tile([TILE, 1], F32, tag="iv")
            nc.sync.dma_start(iv[:], inv_d[r0:r0 + TILE, :])
            scaled = small.tile([TILE, K], F32, tag="scaled")
            nc.scalar.mul(scaled[:], dots[:], iv[:])
            negmax = small.tile([TILE, 1], F32, tag="negmax")
            nc.vector.reduce_max(
                out=negmax[:], in_=scaled[:], axis=mybir.AxisListType.X, negate=True
            )
            expb = small.tile([TILE, K], F32, tag="expb")
            esum = small.tile([TILE, 1], F32, tag="esum")
            nc.scalar.activation(
                out=expb[:], in_=scaled[:], func=mybir.ActivationFunctionType.Exp,
                bias=negmax[:], scale=1.0, accum_out=esum[:],
            )
            rec = small.tile([TILE, 1], F32, tag="rec")
            nc.vector.reciprocal(rec[:], esum[:])

            # unnormalized coefficient matrix Bsel[r, j] = e[j, r%K] when
            # r//K == j%CPG else 0
            bT_ps = psum.tile([K, TILE], F32, tag="bT_ps")
            nc.tensor.transpose(bT_ps[:], expb[:], ident_sb[:])
            bT = small.tile([K, TILE], BF16, tag="bT")
            nc.scalar.copy(bT[:], bT_ps[:])
            brep_ps = psum.tile([TILE, TILE], F32, tag="brep_ps")
            nc.tensor.matmul(brep_ps[:], lhsT=i4t_sb[:], rhs=bT[:])
            bsel = med.tile([TILE, TILE], BF16, tag="bsel")
            nc.vector.tensor_tensor(
                out=bsel[:], in0=brep_ps[:], in1=m128_sb[:],
                op=mybir.AluOpType.mult,
            )

            # even/odd split by gathered-pair parity: par[r, g] applies to
            # Bsel columns j = 4g..4g+3
            parm = small.tile([TILE, K], BF16, tag="parm")
            nc.sync.dma_start(parm[:], par_d[t, :, :])
            parm_ap = parm[:]
            par_bc = bass.AP(
                tensor=parm_ap.tensor,
                offset=parm_ap.offset,
                ap=[parm_ap.ap[0], parm_ap.ap[1], [0, CPG]],
            )
            bselo = med.tile([TILE, TILE], BF16, tag="bselo")
            nc.vector.tensor_tensor(
                out=bselo[:].rearrange("p (g c) -> p g c", c=CPG),
                in0=bsel[:].rearrange("p (g c) -> p g c", c=CPG),
                in1=par_bc,
                op=mybir.AluOpType.mult,
            )
            bsele = med.tile([TILE, TILE], BF16, tag="bsele")
            nc.vector.tensor_tensor(
                out=bsele[:], in0=bsel[:], in1=bselo[:],
                op=mybir.AluOpType.subtract,
            )

            # weighted neighbor reduction:
            # aggT[f, j] = sum_r xg[r, g(j), par*128 + f] * Bsel[r, j]
            aggT_ps = psum.tile([TILE, TILE], F32, tag="aggT_ps")
            for g in range(K):
                cols = slice(CPG * g, CPG * (g + 1))
                nc.tensor.matmul(
                    aggT_ps[:, cols], lhsT=xg[:, g, 0:FN], rhs=bsele[:, cols],
                    start=True, stop=False,
                )
                nc.tensor.matmul(
                    aggT_ps[:, cols], lhsT=xg[:, g, FN:2 * FN], rhs=bselo[:, cols],
                    start=False, stop=True,
                )
            aggT = med.tile([TILE, TILE], BF16, tag="aggT")
            nc.scalar.copy(aggT[:], aggT_ps[:])

            # final projection + softmax normalization:
            # out[n, fo] = (sum_f aggT[f, n] wv[f, fo]) / esum[n]
            out_ps = psum.tile([TILE, FN], F32, tag="out_ps")
            nc.tensor.matmul(out_ps[:], lhsT=aggT[:], rhs=wv_sb[:])
            outs = med.tile([TILE, FN], F32, tag="outs")
            nc.scalar.mul(outs[:], out_ps[:], rec[:])
            nc.sync.dma_start(out_d[r0:r0 + TILE, :], outs[:])

    nc.finalize()
    return nc


def _host_constants():
    r = np.arange(TILE)
    j = np.arange(TILE)
    m128 = (r[:, None] // K == j[None, :] % CPG).astype(np.float32)
    i4t = (np.arange(TILE)[None, :] % K ==
           np.arange(K)[:, None]).astype(ml_dtypes.bfloat16)
    ident = np.eye(TILE, dtype=np.float32)
    return m128, i4t, ident


def _host_prep(inputs):
    nodes = np.ascontiguousarray(np.asarray(inputs["nodes"], dtype=np.float32))
    nlist = np.asarray(inputs["nlist"]).astype(np.int32)
    edges = np.asarray(inputs["edges"], dtype=np.float32)
    inv_degree = np.asarray(inputs["inv_degree"], dtype=np.float32)
    wq = np.asarray(inputs["wq"], dtype=np.float32)
    wk = np.asarray(inputs["wk"], dtype=np.float32)
    wv = np.asarray(inputs["wv"], dtype=np.float32)

    n_table = nodes.shape[0]
    wqkt = np.ascontiguousarray((wq @ wk.T).astype(np.float32))
    m128, i4t, ident = _host_constants()
    pair_view = np.ascontiguousarray(
        nodes.reshape(n_table // 2, 2 * FN).astype(ml_dtypes.bfloat16))

    in_maps = []
    for c in range(NCORES):
        lo = c * SH
        hi = lo + SH

        ed = np.zeros((PAD, K, FE), ml_dtypes.bfloat16)
        ed[:SH] = edges[lo:hi].astype(ml_dtypes.bfloat16)

        xs = np.zeros((PAD, FN), np.float32)
        xs[:SH] = nodes[lo:hi]
        xselfT = np.ascontiguousarray(
            xs.reshape(NT, TILE, FN).transpose(0, 2, 1).astype(ml_dtypes.bfloat16))

        iv = np.ones((PAD, 1), np.float32)
        iv[:SH, 0] = inv_degree[lo:hi]

        nl = np.zeros((PAD, K), np.int32)
        nl[:SH] = nlist[lo:hi]
        # per-tile gather stream: position i holds nlist[t*128 + i//K, i%K];
        # pad rows on the last tile are marked -1 (descriptor-skipped)
        streams = nl.reshape(NT, NIDX).copy()
        streams[NT - 1, (SH - (NT - 1) * TILE) * K:] = -1
        # wrapped int16 pair indices: idxw[t, p%16, s] = stream[t, s*16+p]
        pidx16 = (streams // 2).astype(np.int16).reshape(NT, NW, 16).transpose(0, 2, 1)
        pidx = np.ascontiguousarray(np.tile(pidx16, (1, 8, 1)))   # [NT, 128, NW]
        # parity par[t, r, g] = stream[t, g*128+r] % 2
        par = np.ascontiguousarray(
            (streams % 2).astype(ml_dtypes.bfloat16)
            .reshape(NT, K, TILE).transpose(0, 2, 1)
        )

        in_maps.append({
            "nodes": pair_view,
            "xselfT": xselfT,
            "edges": ed,
            "pidx": pidx,
            "par": par,
            "invdeg": iv,
            "wv": wv.astype(ml_dtypes.bfloat16),
            "wqkt": wqkt.astype(ml_dtypes.bfloat16),
            "m128": m128,
            "i4t": i4t,
            "ident": ident,
        })
    return in_maps


def _run(inputs, trace=False, **kw):
    nc = _CACHE.get("nc")
    if nc is None:
        nc = _build_nc()
        _CACHE["nc"] = nc
    in_maps = _host_prep(inputs)
    res = run_bass_kernel_spmd(
        nc, in_maps, core_ids=list(range(NCORES)), trace=trace, **kw
    )
    out = np.empty((N, FN), np.float32)
    for c in range(NCORES):
        out[c * SH:(c + 1) * SH] = res.results[c]["out"][:SH]
    return out, res


def kernel(**inputs) -> np.ndarray:
    out, _ = _run(inputs, trace=False)
    return out


# revision 15
# speedup vs baseline: 3.4854x; 1.2627x over previous
"""Trainium2 Bass kernel: GNN attention message-passing (AMP layer).

reference math (per node n, K neighbors):
    q      = nodes @ wq                       [N, FE]
    rq     = q @ wk.T = nodes @ (wq @ wk.T)   [N, FE]   (host folds wq@wk.T)
    logit[n,k] = inv_degree[n] * (edges[n,k,:] . rq[n,:])
    b      = softmax_k(logit)
    agg[n] = sum_k b[n,k] * nodes[nlist[n,k]]
    out    = agg @ wv

Distribution: node axis N sharded over 8 cores (6250 rows each, padded to
6272 = 49 tiles of 128). The full nodes table is replicated into every
core's DRAM; the neighbor gather is a per-core dma_gather. No collectives.

The gather uses int16 indices (hardware constraint), which cannot address
50000 rows directly, so the table is viewed as 25000 PAIR tokens of 2x128
floats and idx = nlist//2; the wrong half of each gathered pair is masked
out in the weighted-reduction coefficient matrix (even/odd split).

Per 128-node tile on each core:
  - dma_gather 4096 pair tokens (single_packet=False: >64 desc/engine)
  - logits/softmax on DVE+ACT (per-partition grouped dot products)
  - weighted neighbor reduction as 2x32 small PE matmuls against even/odd
    block-diagonal coefficient matrices built on-chip from softmax output
  - final projection by wv on PE
"""

from contextlib import ExitStack

import ml_dtypes
import numpy as np

import concourse.bass as bass
import concourse.bacc as bacc
import concourse.tile as tile
from concourse import mybir
from concourse.bass_utils import run_bass_kernel_spmd

N, K, FN, FE = 50000, 32, 128, 64
NCORES = 8
SH = N // NCORES            # rows per core (6250)
TILE = 128
NT = -(-SH // TILE)         # tiles per core (49)
PAD = NT * TILE             # padded rows per core (6272)
CPG = TILE // K             # nodes completed per gather block (4)
NIDX = TILE * K             # gathered rows per tile (4096)
NW = NIDX // 16             # wrapped idx columns (256)

F32 = mybir.dt.float32
BF16 = mybir.dt.bfloat16
I16 = mybir.dt.int16

_CACHE: dict = {}


def _build_nc(n_table: int | None = None, nt: int | None = None):
    """Build the SPMD per-core graph. Identical on all 8 cores; only the
    DRAM input contents differ per core."""
    n_table = N if n_table is None else n_table
    nt = NT if nt is None else nt
    pad = nt * TILE
    npair = n_table // 2
    nc = bacc.Bacc(num_swdge_queues=4)

    nodes_d = nc.dram_tensor("nodes", [npair, 2 * FN], BF16, kind="ExternalInput")
    xselfT_d = nc.dram_tensor("xselfT", [nt, FN, TILE], BF16, kind="ExternalInput")
    edges_d = nc.dram_tensor("edges", [pad, K, FE], BF16, kind="ExternalInput")
    pidx_d = nc.dram_tensor("pidx", [nt, 128, NW], I16, kind="ExternalInput")
    par_d = nc.dram_tensor("par", [nt, TILE, K], BF16, kind="ExternalInput")
    inv_d = nc.dram_tensor("invdeg", [pad, 1], F32, kind="ExternalInput")
    wv_d = nc.dram_tensor("wv", [FN, FN], BF16, kind="ExternalInput")
    wqkt_d = nc.dram_tensor("wqkt", [FN, FE], BF16, kind="ExternalInput")
    m128_d = nc.dram_tensor("m128", [TILE, TILE], F32, kind="ExternalInput")
    i4t_d = nc.dram_tensor("i4t", [K, TILE], BF16, kind="ExternalInput")
    ident_d = nc.dram_tensor("ident", [TILE, TILE], F32, kind="ExternalInput")
    out_d = nc.dram_tensor("out", [pad, FN], F32, kind="ExternalOutput")

    with tile.TileContext(nc) as tc, ExitStack() as ctx:
        consts = ctx.enter_context(tc.tile_pool(name="consts", bufs=1))
        big = ctx.enter_context(tc.tile_pool(name="big", bufs=3))
        gat = ctx.enter_context(tc.tile_pool(name="gat", bufs=4))
        idxp = ctx.enter_context(tc.tile_pool(name="idxp", bufs=8))
        med = ctx.enter_context(tc.tile_pool(name="med", bufs=3))
        small = ctx.enter_context(tc.tile_pool(name="small", bufs=4))
        psum = ctx.enter_context(tc.tile_pool(name="psum", bufs=1, space="PSUM"))

        wv_sb = consts.tile([FN, FN], BF16)
        nc.sync.dma_start(wv_sb[:], wv_d[:, :])
        wqkt_sb = consts.tile([FN, FE], BF16)
        nc.sync.dma_start(wqkt_sb[:], wqkt_d[:, :])
        m128_sb = consts.tile([TILE, TILE], F32)
        nc.sync.dma_start(m128_sb[:], m128_d[:, :])
        i4t_sb = consts.tile([K, TILE], BF16)
        nc.sync.dma_start(i4t_sb[:], i4t_d[:, :])
        ident_sb = consts.tile([TILE, TILE], F32)
        nc.sync.dma_start(ident_sb[:], ident_d[:, :])

        valid_last = SH * K - (nt - 1) * NIDX if nt * TILE == PAD else NIDX
        for t in range(nt):
            r0 = t * TILE

            # pair-token gather: element i lands at xg[i%128, i//128, :];
            # the last tile skips its trailing pad rows (-1 indices)
            pidx = idxp.tile([128, NW], I16, tag="pidx")
            nc.sync.dma_start(pidx[:], pidx_d[t, :, :])
            xg = gat.tile([TILE, K, 2 * FN], BF16, tag="xg")
            half = NIDX // 2
            valid = NIDX if t < nt - 1 else valid_last
            for j in range(2):
                reg = min(max(valid - j * half, 0), half)
                nc.gpsimd.dma_gather(
                    xg[:, j * (K // 2):(j + 1) * (K // 2), :],
                    nodes_d[:, :],
                    pidx[:, j * (NW // 2):(j + 1) * (NW // 2)],
                    num_idxs=half, num_idxs_reg=reg, elem_size=2 * FN,
                    single_packet=False, queue_num=(2 * t + j) % 4,
                )

            # self features (pre-transposed on host): xsT[f, n]
            xsT = med.tile([FN, TILE], BF16, tag="xsT")
            nc.sync.dma_start(xsT[:], xselfT_d[t, :, :])

            # rq[n, c] = sum_f xself[n, f] * (wq@wk.T)[f, c]
            rq_ps = psum.tile([TILE, FE], F32, tag="rq_ps")
            nc.tensor.matmul(rq_ps[:], lhsT=xsT[:], rhs=wqkt_sb[:])
            rq = small.tile([TILE, FE], F32, tag="rq")
            nc.scalar.copy(rq[:], rq_ps[:])

            # edges tile + logits: dots[n, k] = sum_c edges[n,k,c] * rq[n,c]
            ed = big.tile([TILE, K, FE], BF16, tag="ed")
            nc.sync.dma_start(ed[:], edges_d[r0:r0 + TILE, :, :])
            prod = big.tile([TILE, K, FE], BF16, tag="prod")
            rq_ap = rq[:]
            rq_bc = bass.AP(
                tensor=rq_ap.tensor,
                offset=rq_ap.offset,
                ap=[rq_ap.ap[0], [0, K], rq_ap.ap[1]],
            )
            nc.vector.tensor_tensor(
                out=prod[:], in0=ed[:], in1=rq_bc, op=mybir.AluOpType.mult
            )
            dots = small.tile([TILE, K], F32, tag="dots")
            nc.vector.tensor_reduce(
                out=dots[:], in_=prod[:], axis=mybir.AxisListType.X,
                op=mybir.AluOpType.add,
            )

            # scale by inv_degree, softmax over k (normalization deferred)
            iv = small.tile([TILE, 1], F32, tag="iv")
            nc.sync.dma_start(iv[:], inv_d[r0:r0 + TILE, :])
            scaled = small.tile([TILE, K], F32, tag="scaled")
            nc.scalar.mul(scaled[:], dots[:], iv[:])
            negmax = small.tile([TILE, 1], F32, tag="negmax")
            nc.vector.reduce_max(
                out=negmax[:], in_=scaled[:], axis=mybir.AxisListType.X, negate=True
            )
            expb = small.tile([TILE, K], F32, tag="expb")
            esum = small.tile([TILE, 1], F32, tag="esum")
            nc.scalar.activation(
                out=expb[:], in_=scaled[:], func=mybir.ActivationFunctionType.Exp,
                bias=negmax[:], scale=1.0, accum_out=esum[:],
            )
            rec = small.tile([TILE, 1], F32, tag="rec")
            nc.vector.reciprocal(rec[:], esum[:])

            # unnormalized coefficient matrix Bsel[r, j] = e[j, r%K] when
            # r//K == j%CPG else 0
            bT_ps = psum.tile([K, TILE], F32, tag="bT_ps")
            nc.tensor.transpose(bT_ps[:], expb[:], ident_sb[:])
            bT = small.tile([K, TILE], BF16, tag="bT")
            nc.scalar.copy(bT[:], bT_ps[:])
            brep_ps = psum.tile([TILE, TILE], F32, tag="brep_ps")
            nc.tensor.matmul(brep_ps[:], lhsT=i4t_sb[:], rhs=bT[:])
            bsel = med.tile([TILE, TILE], BF16, tag="bsel")
            nc.vector.tensor_tensor(
                out=bsel[:], in0=brep_ps[:], in1=m128_sb[:],
                op=mybir.AluOpType.mult,
            )

            # even/odd split by gathered-pair parity: par[r, g] applies to
            # Bsel columns j = 4g..4g+3
            parm = small.tile([TILE, K], BF16, tag="parm")
            nc.sync.dma_start(parm[:], par_d[t, :, :])
            parm_ap = parm[:]
            par_bc = bass.AP(
                tensor=parm_ap.tensor,
                offset=parm_ap.offset,
                ap=[parm_ap.ap[0], parm_ap.ap[1], [0, CPG]],
            )
            bselo = med.tile([TILE, TILE], BF16, tag="bselo")
            nc.vector.tensor_tensor(
                out=bselo[:].rearrange("p (g c) -> p g c", c=CPG),
                in0=bsel[:].rearrange("p (g c) -> p g c", c=CPG),
                in1=par_bc,
                op=mybir.AluOpType.mult,
            )
            bsele = med.tile([TILE, TILE], BF16, tag="bsele")
            nc.vector.tensor_tensor(
                out=bsele[:], in0=bsel[:], in1=bselo[:],
                op=mybir.AluOpType.subtract,
            )

            # weighted neighbor reduction:
            # aggT[f, j] = sum_r xg[r, g(j), par*128 + f] * Bsel[r, j]
            aggT_ps = psum.tile([TILE, TILE], F32, tag="aggT_ps")
            for g in range(K):
                cols = slice(CPG * g, CPG * (g + 1))
                nc.tensor.matmul(
                    aggT_ps[:, cols], lhsT=xg[:, g, 0:FN], rhs=bsele[:, cols],
                    start=True, stop=False,
                )
                nc.tensor.matmul(
                    aggT_ps[:, cols], lhsT=xg[:, g, FN:2 * FN], rhs=bselo[:, cols],
                    start=False, stop=True,
                )
            aggT = med.tile([TILE, TILE], BF16, tag="aggT")
            nc.scalar.copy(aggT[:], aggT_ps[:])

            # final projection + softmax normalization:
            # out[n, fo] = (sum_f aggT[f, n] wv[f, fo]) / esum[n]
            out_ps = psum.tile([TILE, FN], F32, tag="out_ps")
            nc.tensor.matmul(out_ps[:], lhsT=aggT[:], rhs=wv_sb[:])
            outs = med.tile([TILE, FN], F32, tag="outs")
            nc.scalar.mul(outs[:], out_ps[:], rec[:])
            nc.sync.dma_start(out_d[r0:r0 + TILE, :], outs[:])

    nc.finalize()
    return nc


def _host_constants():
    r = np.arange(TILE)
    j = np.arange(TILE)
    m128 = (r[:, None] // K == j[None, :] % CPG).astype(np.float32)
    i4t = (np.arange(TILE)[None, :] % K ==
           np.arange(K)[:, None]).astype(ml_dtypes.bfloat16)
    ident = np.eye(TILE, dtype=np.float32)
    return m128, i4t, ident


def _host_prep(inputs):
    nodes = np.ascontiguousarray(np.asarray(inputs["nodes"], dtype=np.float32))
    nlist = np.asarray(inputs["nlist"]).astype(np.int32)
    edges = np.asarray(inputs["edges"], dtype=np.float32)
    inv_degree = np.asarray(inputs["inv_degree"], dtype=np.float32)
    wq = np.asarray(inputs["wq"], dtype=np.float32)
    wk = np.asarray(inputs["wk"], dtype=np.float32)
    wv = np.asarray(inputs["wv"], dtype=np.float32)

    n_table = nodes.shape[0]
    wqkt = np.ascontiguousarray((wq @ wk.T).astype(np.float32))
    m128, i4t, ident = _host_constants()
    pair_view = np.ascontiguousarray(
        nodes.reshape(n_table // 2, 2 * FN).astype(ml_dtypes.bfloat16))

    in_maps = []
    for c in range(NCORES):
        lo = c * SH
        hi = lo + SH

        ed = np.zeros((PAD, K, FE), ml_dtypes.bfloat16)
        ed[:SH] = edges[lo:hi].astype(ml_dtypes.bfloat16)

        xs = np.zeros((PAD, FN), np.float32)
        xs[:SH] = nodes[lo:hi]
        xselfT = np.ascontiguousarray(
            xs.reshape(NT, TILE, FN).transpose(0, 2, 1).astype(ml_dtypes.bfloat16))

        iv = np.ones((PAD, 1), np.float32)
        iv[:SH, 0] = inv_degree[lo:hi]

        nl = np.zeros((PAD, K), np.int32)
        nl[:SH] = nlist[lo:hi]
        # per-tile gather stream: position i holds nlist[t*128 + i//K, i%K];
        # pad rows on the last tile are marked -1 (descriptor-skipped)
        streams = nl.reshape(NT, NIDX).copy()
        streams[NT - 1, (SH - (NT - 1) * TILE) * K:] = -1
        # wrapped int16 pair indices: idxw[t, p%16, s] = stream[t, s*16+p]
        pidx16 = (streams // 2).astype(np.int16).reshape(NT, NW, 16).transpose(0, 2, 1)
        pidx = np.ascontiguousarray(np.tile(pidx16, (1, 8, 1)))   # [NT, 128, NW]
        # parity par[t, r, g] = stream[t, g*128+r] % 2
        par = np.ascontiguousarray(
            (streams % 2).astype(ml_dtypes.bfloat16)
            .reshape(NT, K, TILE).transpose(0, 2, 1)
        )

        in_maps.append({
            "nodes": pair_view,
            "xselfT": xselfT,
            "edges": ed,
            "pidx": pidx,
            "par": par,
            "invdeg": iv,
            "wv": wv.astype(ml_dtypes.bfloat16),
            "wqkt": wqkt.astype(ml_dtypes.bfloat16),
            "m128": m128,
            "i4t": i4t,
            "ident": ident,
        })
    return in_maps


def _run(inputs, trace=False, **kw):
    nc = _CACHE.get("nc")
    if nc is None:
        nc = _build_nc()
        _CACHE["nc"] = nc
    in_maps = _host_prep(inputs)
    res = run_bass_kernel_spmd(
        nc, in_maps, core_ids=list(range(NCORES)), trace=trace, **kw
    )
    out = np.empty((N, FN), np.float32)
    for c in range(NCORES):
        out[c * SH:(c + 1) * SH] = res.results[c]["out"][:SH]
    return out, res


def kernel(**inputs) -> np.ndarray:
    out, _ = _run(inputs, trace=False)
    return out


# revision 16
# speedup vs baseline: 3.5070x; 1.0062x over previous
"""Trainium2 Bass kernel: GNN attention message-passing (AMP layer).

reference math (per node n, K neighbors):
    q      = nodes @ wq                       [N, FE]
    rq     = q @ wk.T = nodes @ (wq @ wk.T)   [N, FE]   (host folds wq@wk.T)
    logit[n,k] = inv_degree[n] * (edges[n,k,:] . rq[n,:])
    b      = softmax_k(logit)
    agg[n] = sum_k b[n,k] * nodes[nlist[n,k]]
    out    = agg @ wv

Distribution: node axis N sharded over 8 cores (6250 rows each, padded to
6272 = 49 tiles of 128). The full nodes table is replicated into every
core's DRAM; the neighbor gather is a per-core dma_gather. No collectives.

The gather uses int16 indices (hardware constraint), which cannot address
50000 rows directly, so the table is viewed as 25000 PAIR tokens of 2x128
floats and idx = nlist//2; the wrong half of each gathered pair is masked
out in the weighted-reduction coefficient matrix (even/odd split).

Per 128-node tile on each core:
  - dma_gather 4096 pair tokens (single_packet=False: >64 desc/engine)
  - logits/softmax on DVE+ACT (per-partition grouped dot products)
  - weighted neighbor reduction as 2x32 small PE matmuls against even/odd
    block-diagonal coefficient matrices built on-chip from softmax output
  - final projection by wv on PE
"""

from contextlib import ExitStack

import ml_dtypes
import numpy as np

import concourse.bass as bass
import concourse.bacc as bacc
import concourse.tile as tile
from concourse import mybir
from concourse.bass_utils import run_bass_kernel_spmd

N, K, FN, FE = 50000, 32, 128, 64
NCORES = 8
SH = N // NCORES            # rows per core (6250)
TILE = 128
NT = -(-SH // TILE)         # tiles per core (49)
PAD = NT * TILE             # padded rows per core (6272)
CPG = TILE // K             # nodes completed per gather block (4)
NIDX = TILE * K             # gathered rows per tile (4096)
NW = NIDX // 16             # wrapped idx columns (256)

F32 = mybir.dt.float32
BF16 = mybir.dt.bfloat16
I16 = mybir.dt.int16

_CACHE: dict = {}


def _build_nc(n_table: int | None = None, nt: int | None = None):
    """Build the SPMD per-core graph. Identical on all 8 cores; only the
    DRAM input contents differ per core."""
    n_table = N if n_table is None else n_table
    nt = NT if nt is None else nt
    pad = nt * TILE
    npair = n_table // 2
    nc = bacc.Bacc(num_swdge_queues=4)

    nodes_d = nc.dram_tensor("nodes", [npair, 2 * FN], BF16, kind="ExternalInput")
    xselfT_d = nc.dram_tensor("xselfT", [nt, FN, TILE], BF16, kind="ExternalInput")
    edges_d = nc.dram_tensor("edges", [pad, K, FE], BF16, kind="ExternalInput")
    pidx_d = nc.dram_tensor("pidx", [nt, 128, NW], I16, kind="ExternalInput")
    par_d = nc.dram_tensor("par", [nt, TILE, K], BF16, kind="ExternalInput")
    inv_d = nc.dram_tensor("invdeg", [pad, 1], F32, kind="ExternalInput")
    wv_d = nc.dram_tensor("wv", [FN, FN], BF16, kind="ExternalInput")
    wqkt_d = nc.dram_tensor("wqkt", [FN, FE], BF16, kind="ExternalInput")
    m128_d = nc.dram_tensor("m128", [TILE, TILE], F32, kind="ExternalInput")
    i4t_d = nc.dram_tensor("i4t", [K, TILE], BF16, kind="ExternalInput")
    ident_d = nc.dram_tensor("ident", [TILE, TILE], F32, kind="ExternalInput")
    out_d = nc.dram_tensor("out", [pad, FN], F32, kind="ExternalOutput")

    with tile.TileContext(nc) as tc, ExitStack() as ctx:
        consts = ctx.enter_context(tc.tile_pool(name="consts", bufs=1))
        big = ctx.enter_context(tc.tile_pool(name="big", bufs=3))
        gat = ctx.enter_context(tc.tile_pool(name="gat", bufs=4))
        idxp = ctx.enter_context(tc.tile_pool(name="idxp", bufs=8))
        med = ctx.enter_context(tc.tile_pool(name="med", bufs=3))
        small = ctx.enter_context(tc.tile_pool(name="small", bufs=4))
        psum = ctx.enter_context(tc.tile_pool(name="psum", bufs=1, space="PSUM"))

        wv_sb = consts.tile([FN, FN], BF16)
        nc.sync.dma_start(wv_sb[:], wv_d[:, :])
        wqkt_sb = consts.tile([FN, FE], BF16)
        nc.sync.dma_start(wqkt_sb[:], wqkt_d[:, :])
        m128_sb = consts.tile([TILE, TILE], F32)
        nc.sync.dma_start(m128_sb[:], m128_d[:, :])
        i4t_sb = consts.tile([K, TILE], BF16)
        nc.sync.dma_start(i4t_sb[:], i4t_d[:, :])
        ident_sb = consts.tile([TILE, TILE], F32)
        nc.sync.dma_start(ident_sb[:], ident_d[:, :])

        valid_last = SH * K - (nt - 1) * NIDX if nt * TILE == PAD else NIDX
        for t in range(nt):
            r0 = t * TILE

            # pair-token gather: element i lands at xg[i%128, i//128, :];
            # the last tile skips its trailing pad rows (-1 indices)
            pidx = idxp.tile([128, NW], I16, tag="pidx")
            nc.sync.dma_start(pidx[:], pidx_d[t, :, :])
            xg = gat.tile([TILE, K, 2 * FN], BF16, tag="xg")
            quart = NIDX // 4
            valid = NIDX if t < nt - 1 else valid_last
            for j in range(4):
                reg = min(max(valid - j * quart, 0), quart)
                nc.gpsimd.dma_gather(
                    xg[:, j * (K // 4):(j + 1) * (K // 4), :],
                    nodes_d[:, :],
                    pidx[:, j * (NW // 4):(j + 1) * (NW // 4)],
                    num_idxs=quart, num_idxs_reg=reg, elem_size=2 * FN,
                    single_packet=False, queue_num=j,
                )

            # self features (pre-transposed on host): xsT[f, n]
            xsT = med.tile([FN, TILE], BF16, tag="xsT")
            nc.sync.dma_start(xsT[:], xselfT_d[t, :, :])

            # rq[n, c] = sum_f xself[n, f] * (wq@wk.T)[f, c]
            rq_ps = psum.tile([TILE, FE], F32, tag="rq_ps")
            nc.tensor.matmul(rq_ps[:], lhsT=xsT[:], rhs=wqkt_sb[:])
            rq = small.tile([TILE, FE], F32, tag="rq")
            nc.scalar.copy(rq[:], rq_ps[:])

            # edges tile + logits: dots[n, k] = sum_c edges[n,k,c] * rq[n,c]
            ed = big.tile([TILE, K, FE], BF16, tag="ed")
            nc.sync.dma_start(ed[:], edges_d[r0:r0 + TILE, :, :])
            prod = big.tile([TILE, K, FE], BF16, tag="prod")
            rq_ap = rq[:]
            rq_bc = bass.AP(
                tensor=rq_ap.tensor,
                offset=rq_ap.offset,
                ap=[rq_ap.ap[0], [0, K], rq_ap.ap[1]],
            )
            nc.vector.tensor_tensor(
                out=prod[:], in0=ed[:], in1=rq_bc, op=mybir.AluOpType.mult
            )
            dots = small.tile([TILE, K], F32, tag="dots")
            nc.vector.tensor_reduce(
                out=dots[:], in_=prod[:], axis=mybir.AxisListType.X,
                op=mybir.AluOpType.add,
            )

            # scale by inv_degree, softmax over k (normalization deferred)
            iv = small.tile([TILE, 1], F32, tag="iv")
            nc.sync.dma_start(iv[:], inv_d[r0:r0 + TILE, :])
            scaled = small.tile([TILE, K], F32, tag="scaled")
            nc.scalar.mul(scaled[:], dots[:], iv[:])
            negmax = small.tile([TILE, 1], F32, tag="negmax")
            nc.vector.reduce_max(
                out=negmax[:], in_=scaled[:], axis=mybir.AxisListType.X, negate=True
            )
            expb = small.tile([TILE, K], F32, tag="expb")
            esum = small.tile([TILE, 1], F32, tag="esum")
            nc.scalar.activation(
                out=expb[:], in_=scaled[:], func=mybir.ActivationFunctionType.Exp,
                bias=negmax[:], scale=1.0, accum_out=esum[:],
            )
            rec = small.tile([TILE, 1], F32, tag="rec")
            nc.vector.reciprocal(rec[:], esum[:])

            # unnormalized coefficient matrix Bsel[r, j] = e[j, r%K] when
            # r//K == j%CPG else 0
            bT_ps = psum.tile([K, TILE], F32, tag="bT_ps")
            nc.tensor.transpose(bT_ps[:], expb[:], ident_sb[:])
            bT = small.tile([K, TILE], BF16, tag="bT")
            nc.scalar.copy(bT[:], bT_ps[:])
            brep_ps = psum.tile([TILE, TILE], F32, tag="brep_ps")
            nc.tensor.matmul(brep_ps[:], lhsT=i4t_sb[:], rhs=bT[:])
            bsel = med.tile([TILE, TILE], BF16, tag="bsel")
            nc.vector.tensor_tensor(
                out=bsel[:], in0=brep_ps[:], in1=m128_sb[:],
                op=mybir.AluOpType.mult,
            )

            # even/odd split by gathered-pair parity: par[r, g] applies to
            # Bsel columns j = 4g..4g+3
            parm = small.tile([TILE, K], BF16, tag="parm")
            nc.sync.dma_start(parm[:], par_d[t, :, :])
            parm_ap = parm[:]
            par_bc = bass.AP(
                tensor=parm_ap.tensor,
                offset=parm_ap.offset,
                ap=[parm_ap.ap[0], parm_ap.ap[1], [0, CPG]],
            )
            bselo = med.tile([TILE, TILE], BF16, tag="bselo")
            nc.vector.tensor_tensor(
                out=bselo[:].rearrange("p (g c) -> p g c", c=CPG),
                in0=bsel[:].rearrange("p (g c) -> p g c", c=CPG),
                in1=par_bc,
                op=mybir.AluOpType.mult,
            )
            bsele = med.tile([TILE, TILE], BF16, tag="bsele")
            nc.vector.tensor_tensor(
                out=bsele[:], in0=bsel[:], in1=bselo[:],
                op=mybir.AluOpType.subtract,
            )

            # weighted neighbor reduction:
            # aggT[f, j] = sum_r xg[r, g(j), par*128 + f] * Bsel[r, j]
            aggT_ps = psum.tile([TILE, TILE], F32, tag="aggT_ps")
            for g in range(K):
                cols = slice(CPG * g, CPG * (g + 1))
                nc.tensor.matmul(
                    aggT_ps[:, cols], lhsT=xg[:, g, 0:FN], rhs=bsele[:, cols],
                    start=True, stop=False,
                )
                nc.tensor.matmul(
                    aggT_ps[:, cols], lhsT=xg[:, g, FN:2 * FN], rhs=bselo[:, cols],
                    start=False, stop=True,
                )
            aggT = med.tile([TILE, TILE], BF16, tag="aggT")
            nc.scalar.copy(aggT[:], aggT_ps[:])

            # final projection + softmax normalization:
            # out[n, fo] = (sum_f aggT[f, n] wv[f, fo]) / esum[n]
            out_ps = psum.tile([TILE, FN], F32, tag="out_ps")
            nc.tensor.matmul(out_ps[:], lhsT=aggT[:], rhs=wv_sb[:])
            outs = med.tile([TILE, FN], F32, tag="outs")
            nc.scalar.mul(outs[:], out_ps[:], rec[:])
            nc.sync.dma_start(out_d[r0:r0 + TILE, :], outs[:])

    nc.finalize()
    return nc


def _host_constants():
    r = np.arange(TILE)
    j = np.arange(TILE)
    m128 = (r[:, None] // K == j[None, :] % CPG).astype(np.float32)
    i4t = (np.arange(TILE)[None, :] % K ==
           np.arange(K)[:, None]).astype(ml_dtypes.bfloat16)
    ident = np.eye(TILE, dtype=np.float32)
    return m128, i4t, ident


def _host_prep(inputs):
    nodes = np.ascontiguousarray(np.asarray(inputs["nodes"], dtype=np.float32))
    nlist = np.asarray(inputs["nlist"]).astype(np.int32)
    edges = np.asarray(inputs["edges"], dtype=np.float32)
    inv_degree = np.asarray(inputs["inv_degree"], dtype=np.float32)
    wq = np.asarray(inputs["wq"], dtype=np.float32)
    wk = np.asarray(inputs["wk"], dtype=np.float32)
    wv = np.asarray(inputs["wv"], dtype=np.float32)

    n_table = nodes.shape[0]
    wqkt = np.ascontiguousarray((wq @ wk.T).astype(np.float32))
    m128, i4t, ident = _host_constants()
    pair_view = np.ascontiguousarray(
        nodes.reshape(n_table // 2, 2 * FN).astype(ml_dtypes.bfloat16))

    in_maps = []
    for c in range(NCORES):
        lo = c * SH
        hi = lo + SH

        ed = np.zeros((PAD, K, FE), ml_dtypes.bfloat16)
        ed[:SH] = edges[lo:hi].astype(ml_dtypes.bfloat16)

        xs = np.zeros((PAD, FN), np.float32)
        xs[:SH] = nodes[lo:hi]
        xselfT = np.ascontiguousarray(
            xs.reshape(NT, TILE, FN).transpose(0, 2, 1).astype(ml_dtypes.bfloat16))

        iv = np.ones((PAD, 1), np.float32)
        iv[:SH, 0] = inv_degree[lo:hi]

        nl = np.zeros((PAD, K), np.int32)
        nl[:SH] = nlist[lo:hi]
        # per-tile gather stream: position i holds nlist[t*128 + i//K, i%K];
        # pad rows on the last tile are marked -1 (descriptor-skipped)
        streams = nl.reshape(NT, NIDX).copy()
        streams[NT - 1, (SH - (NT - 1) * TILE) * K:] = -1
        # wrapped int16 pair indices: idxw[t, p%16, s] = stream[t, s*16+p]
        pidx16 = (streams // 2).astype(np.int16).reshape(NT, NW, 16).transpose(0, 2, 1)
        pidx = np.ascontiguousarray(np.tile(pidx16, (1, 8, 1)))   # [NT, 128, NW]
        # parity par[t, r, g] = stream[t, g*128+r] % 2
        par = np.ascontiguousarray(
            (streams % 2).astype(ml_dtypes.bfloat16)
            .reshape(NT, K, TILE).transpose(0, 2, 1)
        )

        in_maps.append({
            "nodes": pair_view,
            "xselfT": xselfT,
            "edges": ed,
            "pidx": pidx,
            "par": par,
            "invdeg": iv,
            "wv": wv.astype(ml_dtypes.bfloat16),
            "wqkt": wqkt.astype(ml_dtypes.bfloat16),
            "m128": m128,
            "i4t": i4t,
            "ident": ident,
        })
    return in_maps


def _run(inputs, trace=False, **kw):
    nc = _CACHE.get("nc")
    if nc is None:
        nc = _build_nc()
        _CACHE["nc"] = nc
    in_maps = _host_prep(inputs)
    res = run_bass_kernel_spmd(
        nc, in_maps, core_ids=list(range(NCORES)), trace=trace, **kw
    )
    out = np.empty((N, FN), np.float32)
    for c in range(NCORES):
        out[c * SH:(c + 1) * SH] = res.results[c]["out"][:SH]
    return out, res


def kernel(**inputs) -> np.ndarray:
    out, _ = _run(inputs, trace=False)
    return out


# revision 17
# speedup vs baseline: 3.5139x; 1.0019x over previous
"""Trainium2 Bass kernel: GNN attention message-passing (AMP layer).

reference math (per node n, K neighbors):
    q      = nodes @ wq                       [N, FE]
    rq     = q @ wk.T = nodes @ (wq @ wk.T)   [N, FE]   (host folds wq@wk.T)
    logit[n,k] = inv_degree[n] * (edges[n,k,:] . rq[n,:])
    b      = softmax_k(logit)
    agg[n] = sum_k b[n,k] * nodes[nlist[n,k]]
    out    = agg @ wv

Distribution: node axis N sharded over 8 cores (6250 rows each, padded to
6272 = 49 tiles of 128). The full nodes table is replicated into every
core's DRAM; the neighbor gather is a per-core dma_gather. No collectives.

The gather uses int16 indices (hardware constraint), which cannot address
50000 rows directly, so the table is viewed as 25000 PAIR tokens of 2x128
floats and idx = nlist//2; the wrong half of each gathered pair is masked
out in the weighted-reduction coefficient matrix (even/odd split).

Per 128-node tile on each core:
  - dma_gather 4096 pair tokens (single_packet=False: >64 desc/engine)
  - logits/softmax on DVE+ACT (per-partition grouped dot products)
  - weighted neighbor reduction as 2x32 small PE matmuls against even/odd
    block-diagonal coefficient matrices built on-chip from softmax output
  - final projection by wv on PE
"""

from contextlib import ExitStack

import ml_dtypes
import numpy as np

import concourse.bass as bass
import concourse.bacc as bacc
import concourse.tile as tile
from concourse import mybir
from concourse.bass_utils import run_bass_kernel_spmd

N, K, FN, FE = 50000, 32, 128, 64
NCORES = 8
SH = N // NCORES            # rows per core (6250)
TILE = 128
NT = -(-SH // TILE)         # tiles per core (49)
PAD = NT * TILE             # padded rows per core (6272)
CPG = TILE // K             # nodes completed per gather block (4)
NIDX = TILE * K             # gathered rows per tile (4096)
NW = NIDX // 16             # wrapped idx columns (256)

F32 = mybir.dt.float32
BF16 = mybir.dt.bfloat16
I16 = mybir.dt.int16

_CACHE: dict = {}


def _build_nc(n_table: int | None = None, nt: int | None = None):
    """Build the SPMD per-core graph. Identical on all 8 cores; only the
    DRAM input contents differ per core."""
    n_table = N if n_table is None else n_table
    nt = NT if nt is None else nt
    pad = nt * TILE
    npair = n_table // 2
    nc = bacc.Bacc(num_swdge_queues=4, dynamic_dma_scratch_size=32768)

    nodes_d = nc.dram_tensor("nodes", [npair, 2 * FN], BF16, kind="ExternalInput")
    xselfT_d = nc.dram_tensor("xselfT", [nt, FN, TILE], BF16, kind="ExternalInput")
    edges_d = nc.dram_tensor("edges", [pad, K, FE], BF16, kind="ExternalInput")
    pidx_d = nc.dram_tensor("pidx", [nt, 128, NW], I16, kind="ExternalInput")
    par_d = nc.dram_tensor("par", [nt, TILE, K], BF16, kind="ExternalInput")
    inv_d = nc.dram_tensor("invdeg", [pad, 1], F32, kind="ExternalInput")
    wv_d = nc.dram_tensor("wv", [FN, FN], BF16, kind="ExternalInput")
    wqkt_d = nc.dram_tensor("wqkt", [FN, FE], BF16, kind="ExternalInput")
    m128_d = nc.dram_tensor("m128", [TILE, TILE], F32, kind="ExternalInput")
    i4t_d = nc.dram_tensor("i4t", [K, TILE], BF16, kind="ExternalInput")
    ident_d = nc.dram_tensor("ident", [TILE, TILE], F32, kind="ExternalInput")
    out_d = nc.dram_tensor("out", [pad, FN], F32, kind="ExternalOutput")

    with tile.TileContext(nc) as tc, ExitStack() as ctx:
        consts = ctx.enter_context(tc.tile_pool(name="consts", bufs=1))
        big = ctx.enter_context(tc.tile_pool(name="big", bufs=3))
        gat = ctx.enter_context(tc.tile_pool(name="gat", bufs=4))
        idxp = ctx.enter_context(tc.tile_pool(name="idxp", bufs=8))
        med = ctx.enter_context(tc.tile_pool(name="med", bufs=3))
        small = ctx.enter_context(tc.tile_pool(name="small", bufs=4))
        psum = ctx.enter_context(tc.tile_pool(name="psum", bufs=1, space="PSUM"))

        wv_sb = consts.tile([FN, FN], BF16)
        nc.sync.dma_start(wv_sb[:], wv_d[:, :])
        wqkt_sb = consts.tile([FN, FE], BF16)
        nc.sync.dma_start(wqkt_sb[:], wqkt_d[:, :])
        m128_sb = consts.tile([TILE, TILE], F32)
        nc.sync.dma_start(m128_sb[:], m128_d[:, :])
        i4t_sb = consts.tile([K, TILE], BF16)
        nc.sync.dma_start(i4t_sb[:], i4t_d[:, :])
        ident_sb = consts.tile([TILE, TILE], F32)
        nc.sync.dma_start(ident_sb[:], ident_d[:, :])

        valid_last = SH * K - (nt - 1) * NIDX if nt * TILE == PAD else NIDX
        for t in range(nt):
            r0 = t * TILE

            # pair-token gather: element i lands at xg[i%128, i//128, :];
            # the last tile skips its trailing pad rows (-1 indices)
            pidx = idxp.tile([128, NW], I16, tag="pidx")
            nc.sync.dma_start(pidx[:], pidx_d[t, :, :])
            xg = gat.tile([TILE, K, 2 * FN], BF16, tag="xg")
            quart = NIDX // 4
            valid = NIDX if t < nt - 1 else valid_last
            for j in range(4):
                reg = min(max(valid - j * quart, 0), quart)
                nc.gpsimd.dma_gather(
                    xg[:, j * (K // 4):(j + 1) * (K // 4), :],
                    nodes_d[:, :],
                    pidx[:, j * (NW // 4):(j + 1) * (NW // 4)],
                    num_idxs=quart, num_idxs_reg=reg, elem_size=2 * FN,
                    single_packet=False, queue_num=j,
                )

            # self features (pre-transposed on host): xsT[f, n]
            xsT = med.tile([FN, TILE], BF16, tag="xsT")
            nc.sync.dma_start(xsT[:], xselfT_d[t, :, :])

            # rq[n, c] = sum_f xself[n, f] * (wq@wk.T)[f, c]
            rq_ps = psum.tile([TILE, FE], F32, tag="rq_ps")
            nc.tensor.matmul(rq_ps[:], lhsT=xsT[:], rhs=wqkt_sb[:])
            rq = small.tile([TILE, FE], F32, tag="rq")
            nc.scalar.copy(rq[:], rq_ps[:])

            # edges tile + logits: dots[n, k] = sum_c edges[n,k,c] * rq[n,c]
            ed = big.tile([TILE, K, FE], BF16, tag="ed")
            nc.sync.dma_start(ed[:], edges_d[r0:r0 + TILE, :, :])
            prod = big.tile([TILE, K, FE], BF16, tag="prod")
            rq_ap = rq[:]
            rq_bc = bass.AP(
                tensor=rq_ap.tensor,
                offset=rq_ap.offset,
                ap=[rq_ap.ap[0], [0, K], rq_ap.ap[1]],
            )
            nc.vector.tensor_tensor(
                out=prod[:], in0=ed[:], in1=rq_bc, op=mybir.AluOpType.mult
            )
            dots = small.tile([TILE, K], F32, tag="dots")
            nc.vector.tensor_reduce(
                out=dots[:], in_=prod[:], axis=mybir.AxisListType.X,
                op=mybir.AluOpType.add,
            )

            # scale by inv_degree, softmax over k (normalization deferred)
            iv = small.tile([TILE, 1], F32, tag="iv")
            nc.sync.dma_start(iv[:], inv_d[r0:r0 + TILE, :])
            scaled = small.tile([TILE, K], F32, tag="scaled")
            nc.scalar.mul(scaled[:], dots[:], iv[:])
            negmax = small.tile([TILE, 1], F32, tag="negmax")
            nc.vector.reduce_max(
                out=negmax[:], in_=scaled[:], axis=mybir.AxisListType.X, negate=True
            )
            expb = small.tile([TILE, K], F32, tag="expb")
            esum = small.tile([TILE, 1], F32, tag="esum")
            nc.scalar.activation(
                out=expb[:], in_=scaled[:], func=mybir.ActivationFunctionType.Exp,
                bias=negmax[:], scale=1.0, accum_out=esum[:],
            )
            rec = small.tile([TILE, 1], F32, tag="rec")
            nc.vector.reciprocal(rec[:], esum[:])

            # unnormalized coefficient matrix Bsel[r, j] = e[j, r%K] when
            # r//K == j%CPG else 0
            bT_ps = psum.tile([K, TILE], F32, tag="bT_ps")
            nc.tensor.transpose(bT_ps[:], expb[:], ident_sb[:])
            bT = small.tile([K, TILE], BF16, tag="bT")
            nc.scalar.copy(bT[:], bT_ps[:])
            brep_ps = psum.tile([TILE, TILE], F32, tag="brep_ps")
            nc.tensor.matmul(brep_ps[:], lhsT=i4t_sb[:], rhs=bT[:])
            bsel = med.tile([TILE, TILE], BF16, tag="bsel")
            nc.vector.tensor_tensor(
                out=bsel[:], in0=brep_ps[:], in1=m128_sb[:],
                op=mybir.AluOpType.mult,
            )

            # even/odd split by gathered-pair parity: par[r, g] applies to
            # Bsel columns j = 4g..4g+3
            parm = small.tile([TILE, K], BF16, tag="parm")
            nc.sync.dma_start(parm[:], par_d[t, :, :])
            parm_ap = parm[:]
            par_bc = bass.AP(
                tensor=parm_ap.tensor,
                offset=parm_ap.offset,
                ap=[parm_ap.ap[0], parm_ap.ap[1], [0, CPG]],
            )
            bselo = med.tile([TILE, TILE], BF16, tag="bselo")
            nc.vector.tensor_tensor(
                out=bselo[:].rearrange("p (g c) -> p g c", c=CPG),
                in0=bsel[:].rearrange("p (g c) -> p g c", c=CPG),
                in1=par_bc,
                op=mybir.AluOpType.mult,
            )
            bsele = med.tile([TILE, TILE], BF16, tag="bsele")
            nc.vector.tensor_tensor(
                out=bsele[:], in0=bsel[:], in1=bselo[:],
                op=mybir.AluOpType.subtract,
            )

            # weighted neighbor reduction:
            # aggT[f, j] = sum_r xg[r, g(j), par*128 + f] * Bsel[r, j]
            aggT_ps = psum.tile([TILE, TILE], F32, tag="aggT_ps")
            for g in range(K):
                cols = slice(CPG * g, CPG * (g + 1))
                nc.tensor.matmul(
                    aggT_ps[:, cols], lhsT=xg[:, g, 0:FN], rhs=bsele[:, cols],
                    start=True, stop=False,
                )
                nc.tensor.matmul(
                    aggT_ps[:, cols], lhsT=xg[:, g, FN:2 * FN], rhs=bselo[:, cols],
                    start=False, stop=True,
                )
            aggT = med.tile([TILE, TILE], BF16, tag="aggT")
            nc.scalar.copy(aggT[:], aggT_ps[:])

            # final projection + softmax normalization:
            # out[n, fo] = (sum_f aggT[f, n] wv[f, fo]) / esum[n]
            out_ps = psum.tile([TILE, FN], F32, tag="out_ps")
            nc.tensor.matmul(out_ps[:], lhsT=aggT[:], rhs=wv_sb[:])
            outs = med.tile([TILE, FN], F32, tag="outs")
            nc.scalar.mul(outs[:], out_ps[:], rec[:])
            nc.sync.dma_start(out_d[r0:r0 + TILE, :], outs[:])

    nc.finalize()
    return nc


def _host_constants():
    r = np.arange(TILE)
    j = np.arange(TILE)
    m128 = (r[:, None] // K == j[None, :] % CPG).astype(np.float32)
    i4t = (np.arange(TILE)[None, :] % K ==
           np.arange(K)[:, None]).astype(ml_dtypes.bfloat16)
    ident = np.eye(TILE, dtype=np.float32)
    return m128, i4t, ident


def _host_prep(inputs):
    nodes = np.ascontiguousarray(np.asarray(inputs["nodes"], dtype=np.float32))
    nlist = np.asarray(inputs["nlist"]).astype(np.int32)
    edges = np.asarray(inputs["edges"], dtype=np.float32)
    inv_degree = np.asarray(inputs["inv_degree"], dtype=np.float32)
    wq = np.asarray(inputs["wq"], dtype=np.float32)
    wk = np.asarray(inputs["wk"], dtype=np.float32)
    wv = np.asarray(inputs["wv"], dtype=np.float32)

    n_table = nodes.shape[0]
    wqkt = np.ascontiguousarray((wq @ wk.T).astype(np.float32))
    m128, i4t, ident = _host_constants()
    pair_view = np.ascontiguousarray(
        nodes.reshape(n_table // 2, 2 * FN).astype(ml_dtypes.bfloat16))

    in_maps = []
    for c in range(NCORES):
        lo = c * SH
        hi = lo + SH

        ed = np.zeros((PAD, K, FE), ml_dtypes.bfloat16)
        ed[:SH] = edges[lo:hi].astype(ml_dtypes.bfloat16)

        xs = np.zeros((PAD, FN), np.float32)
        xs[:SH] = nodes[lo:hi]
        xselfT = np.ascontiguousarray(
            xs.reshape(NT, TILE, FN).transpose(0, 2, 1).astype(ml_dtypes.bfloat16))

        iv = np.ones((PAD, 1), np.float32)
        iv[:SH, 0] = inv_degree[lo:hi]

        nl = np.zeros((PAD, K), np.int32)
        nl[:SH] = nlist[lo:hi]
        # per-tile gather stream: position i holds nlist[t*128 + i//K, i%K];
        # pad rows on the last tile are marked -1 (descriptor-skipped)
        streams = nl.reshape(NT, NIDX).copy()
        streams[NT - 1, (SH - (NT - 1) * TILE) * K:] = -1
        # wrapped int16 pair indices: idxw[t, p%16, s] = stream[t, s*16+p]
        pidx16 = (streams // 2).astype(np.int16).reshape(NT, NW, 16).transpose(0, 2, 1)
        pidx = np.ascontiguousarray(np.tile(pidx16, (1, 8, 1)))   # [NT, 128, NW]
        # parity par[t, r, g] = stream[t, g*128+r] % 2
        par = np.ascontiguousarray(
            (streams % 2).astype(ml_dtypes.bfloat16)
            .reshape(NT, K, TILE).transpose(0, 2, 1)
        )

        in_maps.append({
            "nodes": pair_view,
            "xselfT": xselfT,
            "edges": ed,
            "pidx": pidx,
            "par": par,
            "invdeg": iv,
            "wv": wv.astype(ml_dtypes.bfloat16),
            "wqkt": wqkt.astype(ml_dtypes.bfloat16),
            "m128": m128,
            "i4t": i4t,
            "ident": ident,
        })
    return in_maps


def _run(inputs, trace=False, **kw):
    nc = _CACHE.get("nc")
    if nc is None:
        nc = _build_nc()
        _CACHE["nc"] = nc
    in_maps = _host_prep(inputs)
    res = run_bass_kernel_spmd(
        nc, in_maps, core_ids=list(range(NCORES)), trace=trace, **kw
    )
    out = np.empty((N, FN), np.float32)
    for c in range(NCORES):
        out[c * SH:(c + 1) * SH] = res.results[c]["out"][:SH]
    return out, res


def kernel(**inputs) -> np.ndarray:
    out, _ = _run(inputs, trace=False)
    return out
